# revision 10
# baseline (speedup 1.0000x reference)
"""Trainium2 Bass kernel for a spatial self-attention block (fp8 DoubleRow).

reference computation (B=4, H=W=64, C=512, N=H*W=4096):
    h = group_norm(x, gamma, beta, 32 groups)
    q,k,v = h@wq+bq, h@wk+bk, h@wv+bv
    scores = (q @ k^T) / sqrt(C); attn = softmax(scores, -1)
    out = (attn @ v) @ wo + bo + x

Folded-weight form (eliminates the K and V projections entirely):
    G = wq @ wk^T, H = wv @ wo   (host-precomputed)
    scores[i,j] = (s*(h_i G) + s*(bq wk^T)) . x_j   + const_i  (cancels in softmax)
    out_i = (s*(exps_i @ x)) H / denom_i + tH + bv wo + bo + x_i
so the device only computes: group-norm stats -> A-projection (A' = s*(hG)+...)
-> scores = A' . x^T -> AV = exps @ x -> O-projection via H.  The row terms
(tG, tH) are rank-1 corrections computed with tiny matmuls; per-row constants
drop out of softmax.

Sharding: 8 cores = (batch b in 0..3) x (query-half in 0..1); each core
computes stats over its full batch element and attention for its 2048 rows.

All heavy matmuls are fp8(e4m3) DoubleRow ([128, 2, free] APs, contraction
over partition x pair = 256 per pass).  Scaling scheme:
  - G is folded with diag(32*s) input-side into fp8 (WS=32); A' output copy
    applies s/(32*WS) as a per-partition scale column plus the bias column
    s*(tG + bq wk^T)/32.
  - exp uses a fixed shift (cancels in softmax); ex stored fp8.
  - AV accumulates exps @ raw-x; the O-copy scale column is s*AOS/32; H is
    host-scaled by WS into fp8.  After the O-projection the result is
    multiplied by 1/(WS*AOS*denom) and the row R = tH + bv wo + bo enters as
    (R*WS*AOS) x denom rank-1 matmuls.  Residual x is added as f16.
"""

import sys

import numpy as np
import ml_dtypes

if "/opt/trn_rl_repo" not in sys.path:
    sys.path.insert(0, "/opt/trn_rl_repo")

import concourse.mybir as mybir
import concourse.tile as tile
from concourse import bacc
from concourse.bass_utils import run_bass_kernel_spmd

F32 = mybir.dt.float32
F32R = mybir.dt.float32r
F16 = mybir.dt.float16
F8 = mybir.dt.float8e4
AF = mybir.ActivationFunctionType
DR = mybir.MatmulPerfMode.DoubleRow
MULT = mybir.AluOpType.mult
ADD = mybir.AluOpType.add
SUB = mybir.AluOpType.subtract

B, N, C = 4, 4096, 512
HALF = N // 2          # own query rows per core
G, GS = 32, 16         # groups, channels per group
P = 128                # partitions
CO = C // P            # channel subtiles (4)
N_CORES = 8
EPS = 1e-6
SM = 1.0 / float(np.sqrt(C))
WS = 32.0              # weight fp8 scale
SHIFT = 2.0            # exp shift (cancels in softmax)
AOS = 1.0 / 64.0       # attn-output fp8 scale
ICH = 512              # query chunk
NCH = HALF // ICH      # 4
JT = N // P            # 32 key tiles
RT = N // 256          # 16 row-pair tiles (stats)
F8NP = ml_dtypes.float8_e4m3
INV_CNT = 1.0 / (N * GS)


def build_nc():
    nc = bacc.Bacc("TRN2", target_bir_lowering=False, num_devices=N_CORES)

    xT8_d = nc.dram_tensor("xT8", [C, N], F8, kind="ExternalInput")
    x8i_d = nc.dram_tensor("x8i", [RT * P, 2 * C], F8, kind="ExternalInput")
    g16_d = nc.dram_tensor("g16", [P, CO, C], F16, kind="ExternalInput")
    h16_d = nc.dram_tensor("h16", [P, CO, C], F16, kind="ExternalInput")
    h8_d = nc.dram_tensor("h8", [P, CO, C], F8, kind="ExternalInput")
    rows_d = nc.dram_tensor("rows", [1, 4 * C], F32, kind="ExternalInput")
    cst_d = nc.dram_tensor("cst", [P, 4], F32R, kind="ExternalInput")
    xr16_d = nc.dram_tensor("xr16", [HALF, C], F16, kind="ExternalInput")
    out_d = nc.dram_tensor("out", [HALF, C], F16, kind="ExternalOutput")

    xr_t = xr16_d[:].rearrange("(t p) c -> t p c", p=P)   # 16 x [128, 512]
    out_t = out_d[:].rearrange("(t p) c -> t p c", p=P)   # 16 x [128, 512]
    x8i_t = x8i_d[:].rearrange("(t p) c -> t p c", p=P)   # 16 x [128, 1024]

    with tile.TileContext(nc) as tc:
        with (
            tc.tile_pool(name="persist", bufs=1) as persist,
            tc.tile_pool(name="cpool", bufs=1) as cpool,
        ):
            xT8 = persist.tile([P, CO, N], F8, tag="xT8")
            x8i = persist.tile([P, RT, 2, C], F8, tag="x8i")
            apT8 = persist.tile([P, CO, HALF], F8, tag="apT8")

            cst = cpool.tile([P, 4], F32R, tag="cst")
            ones8 = cpool.tile([P, 2, P], F8, tag="ones8")
            g8 = cpool.tile([P, CO, C], F8, tag="g8")
            h8 = cpool.tile([P, CO, C], F8, tag="h8")
            g16 = cpool.tile([P, CO, C], F16, tag="g16")
            h16 = cpool.tile([P, CO, C], F16, tag="h16")
            irows = cpool.tile([1, 4 * C], F32, tag="irows")
            wrow = cpool.tile([1, 2 * C], F32R, tag="wrow")
            grow = cpool.tile([1, 6 * G], F32, tag="grow")
            junk = cpool.tile([1, 2], F32, tag="junk")
            spart = cpool.tile([P, CO], F32, tag="spart")
            scola = cpool.tile([P, CO], F32, tag="scola")
            scolo = cpool.tile([P, CO], F32, tag="scolo")
            t16p = cpool.tile([P, CO], F16, tag="t16p")
            abias = cpool.tile([P, CO], F32, tag="abias")
            rrow = cpool.tile([1, C], F32R, tag="rrow")
            abrow = cpool.tile([1, C], F32R, tag="abrow")

            gamma_row = irows[:, 0 * C:1 * C]
            beta_row = irows[:, 1 * C:2 * C]
            uq_row = irows[:, 2 * C:3 * C]
            rhost_row = irows[:, 3 * C:4 * C]
            s32_row = wrow[:, 0:C]
            tmp_row = wrow[:, C:2 * C]
            g_Sg = grow[:, 0:G]
            g_Qg = grow[:, G:2 * G]
            g_mean = grow[:, 2 * G:3 * G]
            g_tmp = grow[:, 3 * G:4 * G]
            g_var = grow[:, 4 * G:5 * G]
            g_rstd = grow[:, 5 * G:6 * G]

            # tiny configs first (sync ring), bulk on gpsimd ring
            nc.sync.dma_start(cst[:], cst_d[:])
            nc.sync.dma_start(irows[:], rows_d[:])
            nc.gpsimd.memset(ones8[:], 1.0)

            ones_col = cst[:, 0:1]            # F32R
            ones_2 = cst[:, 0:2]              # F32R [128, 2] of ones
            ones_12r = cst[0:1, 0:2]          # F32R [1, 2] of ones
            ones_11f = cst[0:1, 0:1].bitcast(F32)
            shift_col = cst[:, 2:3].bitcast(F32)
            eps_col = cst[:, 3:4].bitcast(F32)

            # preload the sqrt activation-table set (Square/Copy live in every
            # set, so stats squares and the group-norm Sqrt need no reload)
            nc.scalar.activation(junk[:, 0:1], ones_11f, AF.Sqrt)

            # ---- input DMA schedule ----
            # stats inputs first (they gate everything), xT8 next (first
            # halves prioritized for the A-projection), weights after.
            engs = [nc.gpsimd, nc.scalar, nc.sync]
            for c8 in range(8):
                e = engs[c8 % 3]
                e.dma_start(x8i[:, 2 * c8:2 * c8 + 2]
                            .rearrange("p t two c -> p t (two c)"),
                            x8i_t[2 * c8:2 * c8 + 2]
                            .rearrange("t p c -> p t c"))
            nc.gpsimd.dma_start(g16[:], g16_d[:])
            for o in range(CO):       # first halves (A-projection needs these)
                engs[o % 3].dma_start(xT8[:, o, 0:HALF],
                                      xT8_d[o * P:(o + 1) * P, 0:HALF])
            for o in range(CO):
                engs[o % 3].dma_start(xT8[:, o, HALF:N],
                                      xT8_d[o * P:(o + 1) * P, HALF:N])
            nc.scalar.dma_start(h8[:], h8_d[:])
            nc.sync.dma_start(h16[:], h16_d[:])

            # ---- phase 1: group-norm stats ----
            with (
                tc.tile_pool(name="stats_ps", bufs=1, space="PSUM") as stats_ps,
                tc.tile_pool(name="pize_ps", bufs=1, space="PSUM") as pize_ps,
                tc.tile_pool(name="warm_ps", bufs=1, space="PSUM") as warm_ps,
                tc.tile_pool(name="sqpool", bufs=5) as sqpool,
            ):
                def warm(n, tag):
                    w = warm_ps.tile([P, P], F32, tag="warm", name=tag)
                    for wi in range(n):
                        nc.tensor.matmul(w[:], ones8[:], ones8[:],
                                         perf_mode=DR,
                                         start=(wi == 0), stop=(wi == n - 1),
                                         skip_group_check=True)

                warm(24, "w0")
                s_ps = stats_ps.tile([P, C], F32, tag="S")
                q_ps = stats_ps.tile([P, C], F32, tag="Q")
                sq_eng = [nc.scalar, nc.vector, nc.scalar, nc.vector,
                          nc.scalar, nc.vector, nc.gpsimd, nc.scalar,
                          nc.vector, nc.scalar, nc.vector, nc.gpsimd,
                          nc.scalar, nc.vector, nc.scalar, nc.vector]
                sqs = []
                LAG_Q = 3
                for t in range(RT):
                    nc.tensor.matmul(s_ps[:], ones8[:], x8i[:, t],
                                     perf_mode=DR,
                                     start=(t == 0), stop=(t == RT - 1))
                    sq = sqpool.tile([P, 2, C], F8, tag="sq", name=f"sq{t}")
                    sqs.append(sq)
                    e = sq_eng[t]
                    if e is nc.scalar:
                        e.activation(sq[:], x8i[:, t], AF.Square)
                    else:
                        e.tensor_mul(sq[:], x8i[:, t], x8i[:, t])
                    if t >= LAG_Q:
                        nc.tensor.matmul(q_ps[:], ones8[:], sqs[t - LAG_Q],
                                         perf_mode=DR, start=(t == LAG_Q),
                                         stop=False)
                    if t % 4 == 3:
                        warm(4, f"wb{t}")
                for t in range(RT - LAG_Q, RT):
                    nc.tensor.matmul(q_ps[:], ones8[:], sqs[t],
                                     perf_mode=DR, start=False,
                                     stop=(t == RT - 1))

                # group stats chain (vector; Sqrt on scalar, table preloaded)
                nc.vector.reduce_sum(g_Sg,
                                     s_ps[0:1, :].rearrange(
                                         "p (g e) -> p g e", e=GS),
                                     axis=mybir.AxisListType.X)
                nc.vector.tensor_scalar_mul(g_mean, g_Sg, INV_CNT)
                nc.vector.tensor_mul(g_tmp, g_mean, g_mean)
                nc.vector.reduce_sum(g_Qg,
                                     q_ps[0:1, :].rearrange(
                                         "p (g e) -> p g e", e=GS),
                                     axis=mybir.AxisListType.X)
                nc.vector.scalar_tensor_tensor(g_var, g_Qg, INV_CNT, g_tmp,
                                               MULT, SUB)
                nc.scalar.activation(g_rstd, g_var, AF.Sqrt,
                                     bias=eps_col[0:1, :])
                nc.vector.reciprocal(g_rstd, g_rstd)
                # switch scalar table set to exp_and_others off-critical-path
                nc.scalar.activation(junk[:, 1:2], ones_11f, AF.Exp)
                nc.vector.tensor_scalar_mul(g_rstd, g_rstd, WS)  # 32*rstd
                # t path runs on gpsimd in parallel with the s path on vector:
                # tmean = (32*rstd)*mean/32 per group, t = beta - gamma*tmean
                nc.vector.scalar_tensor_tensor(g_tmp, g_rstd, 1.0 / WS,
                                               g_mean, MULT, MULT)
                sv = s32_row.rearrange("p (g e) -> p g e", e=GS)
                tv = tmp_row.rearrange("p (g e) -> p g e", e=GS)
                gv = gamma_row.rearrange("p (g e) -> p g e", e=GS)
                nc.vector.tensor_tensor(
                    sv, gv, g_rstd[:, :, None].to_broadcast((1, G, GS)), MULT)
                nc.gpsimd.tensor_tensor(
                    tv, gv, g_tmp[:, :, None].to_broadcast((1, G, GS)), MULT)
                nc.gpsimd.tensor_sub(tmp_row, beta_row.bitcast(F32R), tmp_row)

                # partition-ize 32*s  ([1,512] row -> [128,4]) on the PE (the
                # pize matmuls are the head of the phase-2 queue; f32r avoids
                # the fp32 two-pass matmul split)
                pp = pize_ps.tile([P, CO, 2], F32, tag="pize", name="pp")
                for o in range(CO):
                    nc.tensor.matmul(pp[:, o, :],
                                     s32_row[0:1, o * P:(o + 1) * P],
                                     ones_12r,
                                     start=(o == 0), stop=(o == CO - 1))
                nc.scalar.activation(spart[:], pp[:, :, 0], AF.Copy)
                nc.vector.tensor_scalar_mul(scola[:], spart[:],
                                            1.0 / (WS * WS))
                nc.vector.tensor_scalar_mul(scolo[:], spart[:], AOS / WS)

                # fold 32*s into fp8 G (scalar + gpsimd; vector stays free)
                for ci in range(CO):
                    if ci % 2 == 0:
                        nc.scalar.activation(g8[:, ci, :], g16[:, ci, :],
                                             AF.Copy,
                                             scale=spart[:, ci:ci + 1])
                    else:
                        nc.gpsimd.tensor_scalar_mul(
                            g8[:, ci, :], g16[:, ci, :],
                            spart[:, ci:ci + 1])

            # ---- phase 2: A-projection + bias rows ----
            cpc = [0]

            def a_copy(ps, o, win):
                e = cpc[0] % 2
                cpc[0] += 1
                dst = apT8[:, o, win * ICH:(win + 1) * ICH]
                if e == 0:
                    nc.scalar.activation(dst, ps[:], AF.Identity,
                                         bias=abias[:, o:o + 1],
                                         scale=scola[:, o:o + 1])
                else:
                    nc.vector.tensor_scalar(dst, ps[:],
                                            scola[:, o:o + 1],
                                            abias[:, o:o + 1], MULT, ADD)

            with (
                tc.tile_pool(name="proj_ps", bufs=5, space="PSUM") as proj_ps,
                tc.tile_pool(name="aux_ps", bufs=1, space="PSUM") as aux_ps,
            ):
                def a_mms(o, win):
                    ps = proj_ps.tile([P, ICH], F32, tag="proj",
                                      name=f"a{o}_{win}")
                    for u in range(2):
                        nc.tensor.matmul(
                            ps[:],
                            g8[:, 2 * u:2 * u + 2, o * P:(o + 1) * P],
                            xT8[:, 2 * u:2 * u + 2,
                                win * ICH:(win + 1) * ICH],
                            perf_mode=DR, start=(u == 0), stop=(u == 1))
                    return ps

                def a_group(o, win):
                    a_copy(a_mms(o, win), o, win)

                # win-major so chunk-0 A columns complete first; win-0 copies
                # are deferred until abias exists (read-before-write hazard)
                ps0 = [a_mms(o, 0) for o in range(CO)]
                # partition-ize t ([1,512] -> [128,4] f16) via tiny matmuls
                pp = aux_ps.tile([P, CO, 2], F32, tag="pize", name="ppt")
                for o in range(CO):
                    nc.tensor.matmul(pp[:, o, :],
                                     tmp_row[0:1, o * P:(o + 1) * P],
                                     ones_12r,
                                     start=(o == 0), stop=(o == CO - 1))
                nc.vector.tensor_copy(t16p[:], pp[:, :, 0])
                # teff = t @ G  (f16), then abias = s*(teff+uq)/32
                bps = aux_ps.tile([1, C], F32, tag="bps")
                for o in range(CO):
                    nc.tensor.matmul(bps[:], t16p[:, o:o + 1], g16[:, o, :],
                                     start=(o == 0), stop=(o == CO - 1))
                nc.vector.tensor_add(abrow[:], bps[:], uq_row)
                nc.vector.scalar_tensor_tensor(abrow[:], abrow[:], 1.0 / WS,
                                               s32_row.bitcast(F32), MULT,
                                               MULT)
                ppa = aux_ps.tile([P, CO, 2], F32, tag="pize", name="ppa")
                for o in range(CO):
                    nc.tensor.matmul(ppa[:, o, :],
                                     abrow[0:1, o * P:(o + 1) * P],
                                     ones_12r,
                                     start=(o == 0), stop=(o == CO - 1))
                nc.vector.tensor_copy(abias[:], ppa[:, :, 0])
                for o in range(CO):
                    a_copy(ps0[o], o, 0)
                for o in range(CO):
                    a_group(o, 1)
                # tH row -> R_used = (tH + bv@wo + bo)*WS*AOS
                ths = aux_ps.tile([1, C], F32, tag="ths")
                for o in range(CO):
                    nc.tensor.matmul(ths[:], t16p[:, o:o + 1], h16[:, o, :],
                                     start=(o == 0), stop=(o == CO - 1))
                nc.vector.scalar_tensor_tensor(rrow[:], ths[:],
                                               WS * AOS, rhost_row, MULT, ADD)
                for win in range(2, NCH):
                    for o in range(CO):
                        a_group(o, win)

            # ---- phase 3: attention + O-projection + residual ----
            with (
                tc.tile_pool(name="av_ps", bufs=1, space="PSUM") as av_ps,
                tc.tile_pool(name="sps_ps", bufs=3, space="PSUM") as sps_ps,
                tc.tile_pool(name="op_ps", bufs=1, space="PSUM") as op_ps,
                tc.tile_pool(name="expp", bufs=5) as expp,
                tc.tile_pool(name="accp", bufs=2) as accp,
                tc.tile_pool(name="aoTp", bufs=2) as aoTp,
                tc.tile_pool(name="drow", bufs=2) as drow,
                tc.tile_pool(name="xres", bufs=6) as xres,
                tc.tile_pool(name="ostage", bufs=2) as ostage,
            ):
                LAG = 3  # AV pairs trail scores by 3 so tail MMs interleave

                def make_tail(ch, avs, acc_a, acc_b, last=False):
                    """Chunk-end work, split into pieces emitted between the
                    next chunk's score matmuls."""
                    st = {}

                    pool, ptag = (sps_ps, "sps") if last else (op_ps, "op")
                    if last:
                        xrs = []
                        for it in range(CO):
                            xr = xres.tile([P, C], F16, tag="xr",
                                           name=f"xrL{it}")
                            nc.sync.dma_start(xr[:], xr_t[ch * CO + it])
                            xrs.append(xr)

                    def p0():
                        nc.vector.tensor_add(acc_a[:], acc_a[:], acc_b[:])
                        dps = pool.tile([2, ICH], F32, tag=ptag,
                                        name=f"dps{ch}")
                        nc.tensor.matmul(dps[:], ones_2,
                                         acc_a[:], start=True, stop=True)
                        d_row = drow.tile([1, ICH], F32R, tag="d_row",
                                          name=f"drow{ch}")
                        nc.vector.tensor_copy(d_row[:], dps[0:1, :])
                        st["d_row"] = d_row

                    def p2():
                        d_row = st["d_row"]
                        dp = pool.tile([P, CO, 2], F32, tag=ptag,
                                       name=f"dp{ch}")
                        for o in range(CO):
                            nc.tensor.matmul(dp[:, o, :],
                                             d_row[0:1, o * P:(o + 1) * P],
                                             ones_12r,
                                             start=(o == 0),
                                             stop=(o == CO - 1))
                        d_inv = drow.tile([P, CO], F32, tag="d_inv",
                                          name=f"dinv{ch}")
                        nc.vector.tensor_scalar_mul(d_inv[:], dp[:, :, 0],
                                                    WS * AOS)
                        nc.vector.reciprocal(d_inv[:], d_inv[:])
                        aoT = aoTp.tile([P, CO, ICH], F8, tag="aoT",
                                        name=f"aoT{ch}")
                        for cs in range(CO):
                            if cs % 2 == 0:
                                nc.vector.tensor_scalar_mul(
                                    aoT[:, cs, :], avs[cs][:],
                                    scolo[:, cs:cs + 1])
                            else:
                                nc.scalar.activation(
                                    aoT[:, cs, :], avs[cs][:], AF.Copy,
                                    scale=scolo[:, cs:cs + 1])
                        st["d_inv"] = d_inv
                        st["aoT"] = aoT

                    def mk_it(it):
                        def p():
                            aoT, d_inv = st["aoT"], st["d_inv"]
                            d_row = st["d_row"]
                            ops = pool.tile([P, C], F32, tag=ptag,
                                            name=f"o{ch}_{it}")
                            for u in range(2):
                                nc.tensor.matmul(
                                    ops[:],
                                    aoT[:, 2 * u:2 * u + 2,
                                        it * P:(it + 1) * P],
                                    h8[:, 2 * u:2 * u + 2, :],
                                    perf_mode=DR, start=(u == 0),
                                    stop=False)
                            nc.tensor.matmul(
                                ops[:],
                                d_row[0:1, it * P:(it + 1) * P],
                                rrow[:], start=False, stop=True)
                            if last:
                                xr = xrs[it]
                            else:
                                xr = xres.tile([P, C], F16, tag="xr",
                                               name=f"xr{ch}_{it}")
                                nc.sync.dma_start(xr[:], xr_t[ch * CO + it])
                            ot = ostage.tile([P, C], F16, tag="ot",
                                             name=f"ot{ch}_{it}")
                            nc.vector.scalar_tensor_tensor(
                                ot[:], ops[:], d_inv[:, it:it + 1], xr[:],
                                MULT, ADD)
                            nc.sync.dma_start(out_t[ch * CO + it], ot[:])
                        return p

                    def noop():
                        pass

                    return [p0, noop, p2, mk_it(0), mk_it(1), mk_it(2),
                            mk_it(3)]

                tail = []
                for ch in range(NCH):
                    i0 = ch * ICH
                    avs = [av_ps.tile([P, ICH], F32, tag=f"av{i}",
                                      name=f"av{ch}_{i}")
                           for i in range(CO)]
                    acc_a = accp.tile([P, ICH], F32R, tag="acc_a",
                                      name=f"acca{ch}")
                    acc_b = accp.tile([P, ICH], F32R, tag="acc_b",
                                      name=f"accb{ch}")

                    def scores(j, ex, jj, i0=i0, acc_a=acc_a, acc_b=acc_b,
                               ch=ch):
                        sps = sps_ps.tile([P, ICH], F32, tag="sps",
                                          name=f"sps{ch}_{j}")
                        for u in range(2):
                            nc.tensor.matmul(
                                sps[:],
                                xT8[:, 2 * u:2 * u + 2, j * P:(j + 1) * P],
                                apT8[:, 2 * u:2 * u + 2, i0:i0 + ICH],
                                perf_mode=DR, start=(u == 0), stop=(u == 1))
                        nc.scalar.activation(ex[:, jj, :], sps[:], AF.Exp,
                                             bias=shift_col, scale=SM)
                        if jj == 0:
                            if j == 0:
                                nc.vector.tensor_copy(acc_a[:], ex[:, 0, :])
                            else:
                                nc.vector.tensor_add(acc_a[:], acc_a[:],
                                                     ex[:, 0, :])
                        else:
                            if j == 1:
                                nc.gpsimd.tensor_copy(acc_b[:], ex[:, 1, :])
                            else:
                                nc.gpsimd.tensor_add(acc_b[:], acc_b[:],
                                                     ex[:, 1, :])

                    def av_mms(t, ex, avs=avs):
                        for cs in range(CO):
                            nc.tensor.matmul(
                                avs[cs][:],
                                x8i[:, t, :, cs * P:(cs + 1) * P],
                                ex[:],
                                perf_mode=DR, start=(t == 0),
                                stop=(t == JT // 2 - 1))

                    lag = 1 if ch == NCH - 1 else LAG
                    exs = {}
                    for t in range(JT // 2):
                        ex = expp.tile([P, 2, ICH], F8, tag="ex",
                                       name=f"ex{ch}_{t}")
                        exs[t] = ex
                        scores(2 * t, ex, 0)
                        scores(2 * t + 1, ex, 1)
                        if 2 <= t <= len(tail) + 1:
                            tail[t - 2]()
                        if t >= lag:
                            av_mms(t - lag, exs.pop(t - lag))
                    for t in range(JT // 2 - lag, JT // 2):
                        av_mms(t, exs.pop(t))
                    tail = make_tail(ch, avs, acc_a, acc_b,
                                     last=(ch == NCH - 1))

                def warm_tail(n, tag):
                    w = sps_ps.tile([P, P], F32, tag="sps", name=tag)
                    for wi in range(n):
                        nc.tensor.matmul(w[:], ones8[:], ones8[:],
                                         perf_mode=DR,
                                         start=(wi == 0), stop=(wi == n - 1),
                                         skip_group_check=True)

                for i, piece in enumerate(tail):
                    piece()
                    if i in (0, 2):
                        warm_tail(8, f"wt{i}")

    nc.compile()
    return nc


_NC = None


def _get_nc():
    global _NC
    if _NC is None:
        _NC = build_nc()
    return _NC


def make_in_maps(x, gn_gamma, gn_beta, wq, bq, wk, bk, wv, bv, wo, bo):
    x4 = np.asarray(x, np.float32).reshape(B, N, C)
    wq, bq = np.asarray(wq, np.float32), np.asarray(bq, np.float32)
    wk, bk = np.asarray(wk, np.float32), np.asarray(bk, np.float32)
    wv, bv = np.asarray(wv, np.float32), np.asarray(bv, np.float32)
    wo, bo = np.asarray(wo, np.float32), np.asarray(bo, np.float32)

    def wlay(w):
        return np.asarray(w, np.float32).reshape(CO, P, C).transpose(1, 0, 2)

    Gm = wq @ wk.T
    Hm = wv @ wo
    uq = bq @ wk.T
    rhost = (bv @ wo + bo) * (WS * AOS)

    rows = np.zeros((1, 4 * C), np.float32)
    for i, v in enumerate((gn_gamma, gn_beta)):
        rows[0, i * C:(i + 1) * C] = np.asarray(v, np.float32)
    rows[0, 2 * C:3 * C] = uq
    rows[0, 3 * C:4 * C] = rhost
    cst = np.zeros((P, 4), np.float32)
    cst[:, 0] = 1.0
    cst[:, 1] = 1.0
    cst[:, 2] = -SHIFT
    cst[:, 3] = EPS
    common = dict(
        g16=np.ascontiguousarray(wlay(Gm).astype(np.float16)),
        h16=np.ascontiguousarray(wlay(Hm).astype(np.float16)),
        h8=np.ascontiguousarray((WS * wlay(Hm)).astype(F8NP)),
        rows=rows, cst=cst,
    )
    in_maps = []
    for c in range(N_CORES):
        b, h = c // 2, c % 2
        own = x4[b, h * HALF:(h + 1) * HALF]
        other = x4[b, (1 - h) * HALF:(2 - h) * HALF]
        xp = np.concatenate([own, other], axis=0)        # [N, C]
        xp8 = xp.astype(F8NP)
        xT8 = np.ascontiguousarray(xp8.T)                # [C, N]
        xi8 = xp8.reshape(RT, 2, P, C).transpose(0, 2, 1, 3) \
                 .reshape(RT * P, 2 * C)
        x8i = np.ascontiguousarray(xi8)
        xr16 = np.ascontiguousarray(own.astype(np.float16))
        in_maps.append(dict(xT8=xT8, x8i=x8i, xr16=xr16, **common))
    return in_maps


def assemble(results):
    out = np.empty((B, N, C), np.float32)
    for c in range(N_CORES):
        b, h = c // 2, c % 2
        out[b, h * HALF:(h + 1) * HALF] = results[c]["out"].astype(np.float32)
    return out.reshape(B, 64, 64, C)


def kernel(**inputs):
    nc = _get_nc()
    in_maps = make_in_maps(**inputs)
    res = run_bass_kernel_spmd(nc, in_maps, list(range(N_CORES)))
    return assemble(res.results)


# revision 11
# speedup vs baseline: 1.0794x; 1.0794x over previous
"""Trainium2 Bass kernel for a spatial self-attention block (fp8 DoubleRow).

reference computation (B=4, H=W=64, C=512, N=H*W=4096):
    h = group_norm(x, gamma, beta, 32 groups)
    q,k,v = h@wq+bq, h@wk+bk, h@wv+bv
    scores = (q @ k^T) / sqrt(C); attn = softmax(scores, -1)
    out = (attn @ v) @ wo + bo + x

Folded-weight form (eliminates the K and V projections entirely):
    G = wq @ wk^T, H = wv @ wo   (host-precomputed)
    scores[i,j] = (s*(h_i G) + s*(bq wk^T)) . x_j   + const_i  (cancels in softmax)
    out_i = (s*(exps_i @ x)) H / denom_i + tH + bv wo + bo + x_i
so the device only computes: group-norm stats -> A-projection (A' = s*(hG)+...)
-> scores = A' . x^T -> AV = exps @ x -> O-projection via H.  The row terms
(tG, tH) are rank-1 corrections computed with tiny matmuls; per-row constants
drop out of softmax.

Sharding: 8 cores = (batch b in 0..3) x (query-half in 0..1); each core
computes stats over its full batch element and attention for its 2048 rows.

All heavy matmuls are fp8(e4m3) DoubleRow ([128, 2, free] APs, contraction
over partition x pair = 256 per pass).  Scaling scheme:
  - G is folded with diag(32*s) input-side into fp8 (WS=32); A' output copy
    applies s/(32*WS) as a per-partition scale column plus the bias column
    s*(tG + bq wk^T)/32.
  - exp uses a fixed shift (cancels in softmax); ex stored fp8.
  - AV accumulates exps @ raw-x; the O-copy scale column is s*AOS/32; H is
    host-scaled by WS into fp8.  After the O-projection the result is
    multiplied by 1/(WS*AOS*denom) and the row R = tH + bv wo + bo enters as
    (R*WS*AOS) x denom rank-1 matmuls.  Residual x is added as f16.
"""

import sys

import numpy as np
import ml_dtypes

if "/opt/trn_rl_repo" not in sys.path:
    sys.path.insert(0, "/opt/trn_rl_repo")

import concourse.mybir as mybir
import concourse.tile as tile
from concourse import bacc
from concourse.bass_utils import run_bass_kernel_spmd

F32 = mybir.dt.float32
F32R = mybir.dt.float32r
F16 = mybir.dt.float16
F8 = mybir.dt.float8e4
AF = mybir.ActivationFunctionType
DR = mybir.MatmulPerfMode.DoubleRow
MULT = mybir.AluOpType.mult
ADD = mybir.AluOpType.add
SUB = mybir.AluOpType.subtract

B, N, C = 4, 4096, 512
HALF = N // 2          # own query rows per core
G, GS = 32, 16         # groups, channels per group
P = 128                # partitions
CO = C // P            # channel subtiles (4)
N_CORES = 8
EPS = 1e-6
SM = 1.0 / float(np.sqrt(C))
WS = 32.0              # weight fp8 scale
SHIFT = 2.0            # exp shift (cancels in softmax)
AOS = 1.0 / 64.0       # attn-output fp8 scale
ICH = 512              # query chunk
NCH = HALF // ICH      # 4
JT = N // P            # 32 key tiles
RT = N // 256          # 16 row-pair tiles (stats)
F8NP = ml_dtypes.float8_e4m3
INV_CNT = 1.0 / (N * GS)


def build_nc():
    nc = bacc.Bacc("TRN2", target_bir_lowering=False, num_devices=N_CORES)

    xT8_d = nc.dram_tensor("xT8", [C, N], F8, kind="ExternalInput")
    x8i_d = nc.dram_tensor("x8i", [RT * P, 2 * C], F8, kind="ExternalInput")
    g16_d = nc.dram_tensor("g16", [P, CO, C], F16, kind="ExternalInput")
    h16_d = nc.dram_tensor("h16", [P, CO, C], F16, kind="ExternalInput")
    h8_d = nc.dram_tensor("h8", [P, CO, C], F8, kind="ExternalInput")
    rows_d = nc.dram_tensor("rows", [1, 4 * C], F32, kind="ExternalInput")
    cst_d = nc.dram_tensor("cst", [P, 4], F32R, kind="ExternalInput")
    xr16_d = nc.dram_tensor("xr16", [HALF, C], F16, kind="ExternalInput")
    out_d = nc.dram_tensor("out", [HALF, C], F16, kind="ExternalOutput")

    xr_t = xr16_d[:].rearrange("(t p) c -> t p c", p=P)   # 16 x [128, 512]
    out_t = out_d[:].rearrange("(t p) c -> t p c", p=P)   # 16 x [128, 512]
    x8i_t = x8i_d[:].rearrange("(t p) c -> t p c", p=P)   # 16 x [128, 1024]

    with tile.TileContext(nc) as tc:
        with (
            tc.tile_pool(name="persist", bufs=1) as persist,
            tc.tile_pool(name="cpool", bufs=1) as cpool,
        ):
            xT8 = persist.tile([P, CO, N], F8, tag="xT8")
            x8i = persist.tile([P, RT, 2, C], F8, tag="x8i")
            apT8 = persist.tile([P, CO, HALF], F8, tag="apT8")

            cst = cpool.tile([P, 4], F32R, tag="cst")
            ones8 = cpool.tile([P, 2, P], F8, tag="ones8")
            g8 = cpool.tile([P, CO, C], F8, tag="g8")
            h8 = cpool.tile([P, CO, C], F8, tag="h8")
            g16 = cpool.tile([P, CO, C], F16, tag="g16")
            h16 = cpool.tile([P, CO, C], F16, tag="h16")
            irows = cpool.tile([1, 4 * C], F32, tag="irows")
            wrow = cpool.tile([1, 2 * C], F32R, tag="wrow")
            grow = cpool.tile([1, 6 * G], F32, tag="grow")
            junk = cpool.tile([1, 2], F32, tag="junk")
            spart = cpool.tile([P, CO], F32, tag="spart")
            scola = cpool.tile([P, CO], F32, tag="scola")
            scolo = cpool.tile([P, CO], F32, tag="scolo")
            t16p = cpool.tile([P, CO], F16, tag="t16p")
            abias = cpool.tile([P, CO], F32, tag="abias")
            rrow = cpool.tile([1, C], F32R, tag="rrow")
            abrow = cpool.tile([1, C], F32R, tag="abrow")

            gamma_row = irows[:, 0 * C:1 * C]
            beta_row = irows[:, 1 * C:2 * C]
            uq_row = irows[:, 2 * C:3 * C]
            rhost_row = irows[:, 3 * C:4 * C]
            s32_row = wrow[:, 0:C]
            tmp_row = wrow[:, C:2 * C]
            g_Sg = grow[:, 0:G]
            g_Qg = grow[:, G:2 * G]
            g_mean = grow[:, 2 * G:3 * G]
            g_tmp = grow[:, 3 * G:4 * G]
            g_var = grow[:, 4 * G:5 * G]
            g_rstd = grow[:, 5 * G:6 * G]

            # tiny configs first (sync ring), bulk on gpsimd ring
            nc.sync.dma_start(cst[:], cst_d[:])
            nc.sync.dma_start(irows[:], rows_d[:])
            nc.gpsimd.memset(ones8[:], 1.0)

            ones_col = cst[:, 0:1]            # F32R
            ones_2 = cst[:, 0:2]              # F32R [128, 2] of ones
            ones_12r = cst[0:1, 0:2]          # F32R [1, 2] of ones
            ones_11f = cst[0:1, 0:1].bitcast(F32)
            shift_col = cst[:, 2:3].bitcast(F32)
            eps_col = cst[:, 3:4].bitcast(F32)

            # preload the sqrt activation-table set (Square/Copy live in every
            # set, so stats squares and the group-norm Sqrt need no reload)
            nc.scalar.activation(junk[:, 0:1], ones_11f, AF.Sqrt)

            # ---- input DMA schedule ----
            # stats inputs first (they gate everything), xT8 next (first
            # halves prioritized for the A-projection), weights after.
            engs = [nc.gpsimd, nc.scalar, nc.sync]
            for c8 in range(8):
                e = engs[c8 % 3]
                e.dma_start(x8i[:, 2 * c8:2 * c8 + 2]
                            .rearrange("p t two c -> p t (two c)"),
                            x8i_t[2 * c8:2 * c8 + 2]
                            .rearrange("t p c -> p t c"))
            nc.gpsimd.dma_start(g16[:], g16_d[:])
            for o in range(CO):       # first halves (A-projection needs these)
                engs[o % 3].dma_start(xT8[:, o, 0:HALF],
                                      xT8_d[o * P:(o + 1) * P, 0:HALF])
            for o in range(CO):
                engs[o % 3].dma_start(xT8[:, o, HALF:N],
                                      xT8_d[o * P:(o + 1) * P, HALF:N])
            nc.scalar.dma_start(h8[:], h8_d[:])
            nc.sync.dma_start(h16[:], h16_d[:])

            # ---- phase 1: group-norm stats ----
            with (
                tc.tile_pool(name="stats_ps", bufs=1, space="PSUM") as stats_ps,
                tc.tile_pool(name="pize_ps", bufs=1, space="PSUM") as pize_ps,
                tc.tile_pool(name="warm_ps", bufs=1, space="PSUM") as warm_ps,
                tc.tile_pool(name="sqpool", bufs=6) as sqpool,
            ):
                def warm(n, tag):
                    w = warm_ps.tile([P, P], F32, tag="warm", name=tag)
                    for wi in range(n):
                        nc.tensor.matmul(w[:], ones8[:], ones8[:],
                                         perf_mode=DR,
                                         start=(wi == 0), stop=(wi == n - 1),
                                         skip_group_check=True)

                warm(24, "w0")
                s_ps = stats_ps.tile([P, C], F32, tag="S")
                q_ps = stats_ps.tile([P, C], F32, tag="Q")
                sq_eng = [nc.scalar, nc.vector, nc.scalar, nc.vector,
                          nc.scalar, nc.vector, nc.scalar, nc.scalar,
                          nc.vector, nc.scalar, nc.vector, nc.scalar,
                          nc.scalar, nc.vector, nc.scalar, nc.scalar]
                sqs = []
                LAG_Q = 4
                for t in range(RT):
                    nc.tensor.matmul(s_ps[:], ones8[:], x8i[:, t],
                                     perf_mode=DR,
                                     start=(t == 0), stop=(t == RT - 1))
                    sq = sqpool.tile([P, 2, C], F8, tag="sq", name=f"sq{t}")
                    sqs.append(sq)
                    e = sq_eng[t]
                    if e is nc.scalar:
                        e.activation(sq[:], x8i[:, t], AF.Square)
                    else:
                        e.tensor_mul(sq[:], x8i[:, t], x8i[:, t])
                    if t >= LAG_Q:
                        nc.tensor.matmul(q_ps[:], ones8[:], sqs[t - LAG_Q],
                                         perf_mode=DR, start=(t == LAG_Q),
                                         stop=False)
                    if t % 4 == 3:
                        warm(4, f"wb{t}")
                for t in range(RT - LAG_Q, RT):
                    nc.tensor.matmul(q_ps[:], ones8[:], sqs[t],
                                     perf_mode=DR, start=False,
                                     stop=(t == RT - 1))

                # group stats chain (vector; Sqrt on scalar, table preloaded)
                nc.vector.reduce_sum(g_Sg,
                                     s_ps[0:1, :].rearrange(
                                         "p (g e) -> p g e", e=GS),
                                     axis=mybir.AxisListType.X)
                nc.vector.tensor_scalar_mul(g_mean, g_Sg, INV_CNT)
                nc.vector.tensor_mul(g_tmp, g_mean, g_mean)
                nc.vector.reduce_sum(g_Qg,
                                     q_ps[0:1, :].rearrange(
                                         "p (g e) -> p g e", e=GS),
                                     axis=mybir.AxisListType.X)
                nc.vector.scalar_tensor_tensor(g_var, g_Qg, INV_CNT, g_tmp,
                                               MULT, SUB)
                nc.scalar.activation(g_rstd, g_var, AF.Sqrt,
                                     bias=eps_col[0:1, :])
                nc.vector.reciprocal(g_rstd, g_rstd)
                # switch scalar table set to exp_and_others off-critical-path
                nc.scalar.activation(junk[:, 1:2], ones_11f, AF.Exp)
                nc.vector.tensor_scalar_mul(g_rstd, g_rstd, WS)  # 32*rstd
                # t path runs on gpsimd in parallel with the s path on vector:
                # tmean = (32*rstd)*mean/32 per group, t = beta - gamma*tmean
                nc.vector.scalar_tensor_tensor(g_tmp, g_rstd, 1.0 / WS,
                                               g_mean, MULT, MULT)
                sv = s32_row.rearrange("p (g e) -> p g e", e=GS)
                tv = tmp_row.rearrange("p (g e) -> p g e", e=GS)
                gv = gamma_row.rearrange("p (g e) -> p g e", e=GS)
                nc.vector.tensor_tensor(
                    sv, gv, g_rstd[:, :, None].to_broadcast((1, G, GS)), MULT)
                nc.vector.tensor_tensor(
                    tv, gv, g_tmp[:, :, None].to_broadcast((1, G, GS)), MULT)
                nc.vector.tensor_sub(tmp_row, beta_row.bitcast(F32R), tmp_row)

                # partition-ize 32*s  ([1,512] row -> [128,4]) on the PE (the
                # pize matmuls are the head of the phase-2 queue; f32r avoids
                # the fp32 two-pass matmul split)
                pp = pize_ps.tile([P, CO, 2], F32, tag="pize", name="pp")
                for o in range(CO):
                    nc.tensor.matmul(pp[:, o, :],
                                     s32_row[0:1, o * P:(o + 1) * P],
                                     ones_12r,
                                     start=(o == 0), stop=(o == CO - 1))
                nc.scalar.activation(spart[:], pp[:, :, 0], AF.Copy)
                nc.vector.tensor_scalar_mul(scola[:], spart[:],
                                            1.0 / (WS * WS))
                nc.vector.tensor_scalar_mul(scolo[:], spart[:], AOS / WS)

                # fold 32*s into fp8 G (scalar + vector; gpsimd is slow on
                # f16 and its SBUF traffic stalls the DVE port)
                for ci in range(CO):
                    if ci % 2 == 0:
                        nc.scalar.activation(g8[:, ci, :], g16[:, ci, :],
                                             AF.Copy,
                                             scale=spart[:, ci:ci + 1])
                    else:
                        nc.vector.tensor_scalar_mul(
                            g8[:, ci, :], g16[:, ci, :],
                            spart[:, ci:ci + 1])

            # ---- phase 2: A-projection + bias rows ----
            cpc = [0]

            def a_copy(ps, o, win):
                e = cpc[0] % 2
                cpc[0] += 1
                dst = apT8[:, o, win * ICH:(win + 1) * ICH]
                if e == 0:
                    nc.scalar.activation(dst, ps[:], AF.Identity,
                                         bias=abias[:, o:o + 1],
                                         scale=scola[:, o:o + 1])
                else:
                    nc.vector.tensor_scalar(dst, ps[:],
                                            scola[:, o:o + 1],
                                            abias[:, o:o + 1], MULT, ADD)

            with (
                tc.tile_pool(name="proj_ps", bufs=5, space="PSUM") as proj_ps,
                tc.tile_pool(name="aux_ps", bufs=1, space="PSUM") as aux_ps,
            ):
                def a_mms(o, win):
                    ps = proj_ps.tile([P, ICH], F32, tag="proj",
                                      name=f"a{o}_{win}")
                    for u in range(2):
                        nc.tensor.matmul(
                            ps[:],
                            g8[:, 2 * u:2 * u + 2, o * P:(o + 1) * P],
                            xT8[:, 2 * u:2 * u + 2,
                                win * ICH:(win + 1) * ICH],
                            perf_mode=DR, start=(u == 0), stop=(u == 1))
                    return ps

                def a_group(o, win):
                    a_copy(a_mms(o, win), o, win)

                # win-major so chunk-0 A columns complete first; win-0 copies
                # are deferred until abias exists (read-before-write hazard)
                ps0 = [a_mms(o, 0) for o in range(CO)]
                # partition-ize t ([1,512] -> [128,4] f16) via tiny matmuls
                pp = aux_ps.tile([P, CO, 2], F32, tag="pize", name="ppt")
                for o in range(CO):
                    nc.tensor.matmul(pp[:, o, :],
                                     tmp_row[0:1, o * P:(o + 1) * P],
                                     ones_12r,
                                     start=(o == 0), stop=(o == CO - 1))
                nc.vector.tensor_copy(t16p[:], pp[:, :, 0])
                # teff = t @ G  (f16), then abias = s*(teff+uq)/32
                bps = aux_ps.tile([1, C], F32, tag="bps")
                for o in range(CO):
                    nc.tensor.matmul(bps[:], t16p[:, o:o + 1], g16[:, o, :],
                                     start=(o == 0), stop=(o == CO - 1))
                nc.vector.tensor_add(abrow[:], bps[:], uq_row)
                nc.vector.scalar_tensor_tensor(abrow[:], abrow[:], 1.0 / WS,
                                               s32_row.bitcast(F32), MULT,
                                               MULT)
                ppa = aux_ps.tile([P, CO, 2], F32, tag="pize", name="ppa")
                for o in range(CO):
                    nc.tensor.matmul(ppa[:, o, :],
                                     abrow[0:1, o * P:(o + 1) * P],
                                     ones_12r,
                                     start=(o == 0), stop=(o == CO - 1))
                nc.vector.tensor_copy(abias[:], ppa[:, :, 0])
                for o in range(CO):
                    a_copy(ps0[o], o, 0)
                for o in range(CO):
                    a_group(o, 1)
                # tH row -> R_used = (tH + bv@wo + bo)*WS*AOS
                ths = aux_ps.tile([1, C], F32, tag="ths")
                for o in range(CO):
                    nc.tensor.matmul(ths[:], t16p[:, o:o + 1], h16[:, o, :],
                                     start=(o == 0), stop=(o == CO - 1))
                nc.vector.scalar_tensor_tensor(rrow[:], ths[:],
                                               WS * AOS, rhost_row, MULT, ADD)
                for win in range(2, NCH):
                    for o in range(CO):
                        a_group(o, win)

            # ---- phase 3: attention + O-projection + residual ----
            with (
                tc.tile_pool(name="av_ps", bufs=1, space="PSUM") as av_ps,
                tc.tile_pool(name="sps_ps", bufs=3, space="PSUM") as sps_ps,
                tc.tile_pool(name="op_ps", bufs=1, space="PSUM") as op_ps,
                tc.tile_pool(name="expp", bufs=5) as expp,
                tc.tile_pool(name="accp", bufs=2) as accp,
                tc.tile_pool(name="aoTp", bufs=2) as aoTp,
                tc.tile_pool(name="drow", bufs=2) as drow,
                tc.tile_pool(name="xres", bufs=6) as xres,
                tc.tile_pool(name="ostage", bufs=2) as ostage,
            ):
                LAG = 3  # AV pairs trail scores by 3 so tail MMs interleave

                def make_tail(ch, avs, acc_a, acc_b, last=False):
                    """Chunk-end work, split into pieces emitted between the
                    next chunk's score matmuls."""
                    st = {}

                    pool, ptag = (sps_ps, "sps") if last else (op_ps, "op")
                    if last:
                        xrs = []
                        for it in range(CO):
                            xr = xres.tile([P, C], F16, tag="xr",
                                           name=f"xrL{it}")
                            nc.sync.dma_start(xr[:], xr_t[ch * CO + it])
                            xrs.append(xr)

                    def p0():
                        nc.vector.tensor_add(acc_a[:], acc_a[:], acc_b[:])
                        dps = pool.tile([1, ICH], F32, tag=ptag,
                                        name=f"dps{ch}")
                        nc.tensor.matmul(dps[:], ones_col.bitcast(F32),
                                         acc_a[:], start=True, stop=True)
                        d_row = drow.tile([1, ICH], F32R, tag="d_row",
                                          name=f"drow{ch}")
                        nc.vector.tensor_copy(d_row[:], dps[:])
                        st["d_row"] = d_row

                    def p2():
                        d_row = st["d_row"]
                        dp = pool.tile([P, CO, 2], F32, tag=ptag,
                                       name=f"dp{ch}")
                        for o in range(CO):
                            nc.tensor.matmul(dp[:, o, :],
                                             d_row[0:1, o * P:(o + 1) * P],
                                             ones_12r,
                                             start=(o == 0),
                                             stop=(o == CO - 1))
                        d_inv = drow.tile([P, CO], F32, tag="d_inv",
                                          name=f"dinv{ch}")
                        nc.vector.tensor_scalar_mul(d_inv[:], dp[:, :, 0],
                                                    WS * AOS)
                        nc.vector.reciprocal(d_inv[:], d_inv[:])
                        aoT = aoTp.tile([P, CO, ICH], F8, tag="aoT",
                                        name=f"aoT{ch}")
                        for cs in range(CO):
                            if cs % 2 == 0:
                                nc.vector.tensor_scalar_mul(
                                    aoT[:, cs, :], avs[cs][:],
                                    scolo[:, cs:cs + 1])
                            else:
                                nc.scalar.activation(
                                    aoT[:, cs, :], avs[cs][:], AF.Copy,
                                    scale=scolo[:, cs:cs + 1])
                        st["d_inv"] = d_inv
                        st["aoT"] = aoT

                    def mk_it(it):
                        def p():
                            aoT, d_inv = st["aoT"], st["d_inv"]
                            d_row = st["d_row"]
                            ops = pool.tile([P, C], F32, tag=ptag,
                                            name=f"o{ch}_{it}")
                            for u in range(2):
                                nc.tensor.matmul(
                                    ops[:],
                                    aoT[:, 2 * u:2 * u + 2,
                                        it * P:(it + 1) * P],
                                    h8[:, 2 * u:2 * u + 2, :],
                                    perf_mode=DR, start=(u == 0),
                                    stop=False)
                            nc.tensor.matmul(
                                ops[:],
                                d_row[0:1, it * P:(it + 1) * P],
                                rrow[:], start=False, stop=True)
                            if last:
                                xr = xrs[it]
                            else:
                                xr = xres.tile([P, C], F16, tag="xr",
                                               name=f"xr{ch}_{it}")
                                nc.sync.dma_start(xr[:], xr_t[ch * CO + it])
                            ot = ostage.tile([P, C], F16, tag="ot",
                                             name=f"ot{ch}_{it}")
                            nc.vector.scalar_tensor_tensor(
                                ot[:], ops[:], d_inv[:, it:it + 1], xr[:],
                                MULT, ADD)
                            nc.sync.dma_start(out_t[ch * CO + it], ot[:])
                        return p

                    def noop():
                        pass

                    return [p0, noop, p2, mk_it(0), mk_it(1), mk_it(2),
                            mk_it(3)]

                tail = []
                for ch in range(NCH):
                    i0 = ch * ICH
                    avs = [av_ps.tile([P, ICH], F32, tag=f"av{i}",
                                      name=f"av{ch}_{i}")
                           for i in range(CO)]
                    acc_a = accp.tile([P, ICH], F32, tag="acc_a",
                                      name=f"acca{ch}")
                    acc_b = accp.tile([P, ICH], F32, tag="acc_b",
                                      name=f"accb{ch}")

                    def scores(j, ex, jj, i0=i0, acc_a=acc_a, acc_b=acc_b,
                               ch=ch):
                        sps = sps_ps.tile([P, ICH], F32, tag="sps",
                                          name=f"sps{ch}_{j}")
                        for u in range(2):
                            nc.tensor.matmul(
                                sps[:],
                                xT8[:, 2 * u:2 * u + 2, j * P:(j + 1) * P],
                                apT8[:, 2 * u:2 * u + 2, i0:i0 + ICH],
                                perf_mode=DR, start=(u == 0), stop=(u == 1))
                        nc.scalar.activation(ex[:, jj, :], sps[:], AF.Exp,
                                             bias=shift_col, scale=SM)
                        if jj == 0:
                            if j == 0:
                                nc.vector.tensor_copy(acc_a[:], ex[:, 0, :])
                            else:
                                nc.vector.tensor_add(acc_a[:], acc_a[:],
                                                     ex[:, 0, :])
                        else:
                            if j == 1:
                                nc.gpsimd.tensor_copy(acc_b[:], ex[:, 1, :])
                            else:
                                nc.gpsimd.tensor_add(acc_b[:], acc_b[:],
                                                     ex[:, 1, :])

                    def av_mms(t, ex, avs=avs):
                        for cs in range(CO):
                            nc.tensor.matmul(
                                avs[cs][:],
                                x8i[:, t, :, cs * P:(cs + 1) * P],
                                ex[:],
                                perf_mode=DR, start=(t == 0),
                                stop=(t == JT // 2 - 1))

                    lag = 1 if ch == NCH - 1 else LAG
                    exs = {}
                    for t in range(JT // 2):
                        ex = expp.tile([P, 2, ICH], F8, tag="ex",
                                       name=f"ex{ch}_{t}")
                        exs[t] = ex
                        scores(2 * t, ex, 0)
                        scores(2 * t + 1, ex, 1)
                        if 2 <= t <= len(tail) + 1:
                            tail[t - 2]()
                        if t >= lag:
                            av_mms(t - lag, exs.pop(t - lag))
                    for t in range(JT // 2 - lag, JT // 2):
                        av_mms(t, exs.pop(t))
                    tail = make_tail(ch, avs, acc_a, acc_b,
                                     last=(ch == NCH - 1))

                def warm_tail(n, tag):
                    w = sps_ps.tile([P, P], F32, tag="sps", name=tag)
                    for wi in range(n):
                        nc.tensor.matmul(w[:], ones8[:], ones8[:],
                                         perf_mode=DR,
                                         start=(wi == 0), stop=(wi == n - 1),
                                         skip_group_check=True)

                for i, piece in enumerate(tail):
                    piece()
                    if i in (0, 2):
                        warm_tail(8, f"wt{i}")

    nc.compile()
    return nc


_NC = None


def _get_nc():
    global _NC
    if _NC is None:
        _NC = build_nc()
    return _NC


def make_in_maps(x, gn_gamma, gn_beta, wq, bq, wk, bk, wv, bv, wo, bo):
    x4 = np.asarray(x, np.float32).reshape(B, N, C)
    wq, bq = np.asarray(wq, np.float32), np.asarray(bq, np.float32)
    wk, bk = np.asarray(wk, np.float32), np.asarray(bk, np.float32)
    wv, bv = np.asarray(wv, np.float32), np.asarray(bv, np.float32)
    wo, bo = np.asarray(wo, np.float32), np.asarray(bo, np.float32)

    def wlay(w):
        return np.asarray(w, np.float32).reshape(CO, P, C).transpose(1, 0, 2)

    Gm = wq @ wk.T
    Hm = wv @ wo
    uq = bq @ wk.T
    rhost = (bv @ wo + bo) * (WS * AOS)

    rows = np.zeros((1, 4 * C), np.float32)
    for i, v in enumerate((gn_gamma, gn_beta)):
        rows[0, i * C:(i + 1) * C] = np.asarray(v, np.float32)
    rows[0, 2 * C:3 * C] = uq
    rows[0, 3 * C:4 * C] = rhost
    cst = np.zeros((P, 4), np.float32)
    cst[:, 0] = 1.0
    cst[:, 1] = 1.0
    cst[:, 2] = -SHIFT
    cst[:, 3] = EPS
    common = dict(
        g16=np.ascontiguousarray(wlay(Gm).astype(np.float16)),
        h16=np.ascontiguousarray(wlay(Hm).astype(np.float16)),
        h8=np.ascontiguousarray((WS * wlay(Hm)).astype(F8NP)),
        rows=rows, cst=cst,
    )
    in_maps = []
    for c in range(N_CORES):
        b, h = c // 2, c % 2
        own = x4[b, h * HALF:(h + 1) * HALF]
        other = x4[b, (1 - h) * HALF:(2 - h) * HALF]
        xp = np.concatenate([own, other], axis=0)        # [N, C]
        xp8 = xp.astype(F8NP)
        xT8 = np.ascontiguousarray(xp8.T)                # [C, N]
        xi8 = xp8.reshape(RT, 2, P, C).transpose(0, 2, 1, 3) \
                 .reshape(RT * P, 2 * C)
        x8i = np.ascontiguousarray(xi8)
        xr16 = np.ascontiguousarray(own.astype(np.float16))
        in_maps.append(dict(xT8=xT8, x8i=x8i, xr16=xr16, **common))
    return in_maps


def assemble(results):
    out = np.empty((B, N, C), np.float32)
    for c in range(N_CORES):
        b, h = c // 2, c % 2
        out[b, h * HALF:(h + 1) * HALF] = results[c]["out"].astype(np.float32)
    return out.reshape(B, 64, 64, C)


def kernel(**inputs):
    nc = _get_nc()
    in_maps = make_in_maps(**inputs)
    res = run_bass_kernel_spmd(nc, in_maps, list(range(N_CORES)))
    return assemble(res.results)


# revision 12
# speedup vs baseline: 1.1033x; 1.0221x over previous
"""Trainium2 Bass kernel for a spatial self-attention block (fp8 DoubleRow).

reference computation (B=4, H=W=64, C=512, N=H*W=4096):
    h = group_norm(x, gamma, beta, 32 groups)
    q,k,v = h@wq+bq, h@wk+bk, h@wv+bv
    scores = (q @ k^T) / sqrt(C); attn = softmax(scores, -1)
    out = (attn @ v) @ wo + bo + x

Folded-weight form (eliminates the K and V projections entirely):
    G = wq @ wk^T, H = wv @ wo   (host-precomputed)
    scores[i,j] = (s*(h_i G) + s*(bq wk^T)) . x_j   + const_i  (cancels in softmax)
    out_i = (s*(exps_i @ x)) H / denom_i + tH + bv wo + bo + x_i
so the device only computes: group-norm stats -> A-projection (A' = s*(hG)+...)
-> scores = A' . x^T -> AV = exps @ x -> O-projection via H.  The row terms
(tG, tH) are rank-1 corrections computed with tiny matmuls; per-row constants
drop out of softmax.

Sharding: 8 cores = (batch b in 0..3) x (query-half in 0..1); each core
computes stats over its full batch element and attention for its 2048 rows.

All heavy matmuls are fp8(e4m3) DoubleRow ([128, 2, free] APs, contraction
over partition x pair = 256 per pass).  Scaling scheme:
  - G is folded with diag(32*s) input-side into fp8 (WS=32); A' output copy
    applies s/(32*WS) as a per-partition scale column plus the bias column
    s*(tG + bq wk^T)/32.
  - exp uses a fixed shift (cancels in softmax); ex stored fp8.
  - AV accumulates exps @ raw-x; the O-copy scale column is s*AOS/32; H is
    host-scaled by WS into fp8.  After the O-projection the result is
    multiplied by 1/(WS*AOS*denom) and the row R = tH + bv wo + bo enters as
    (R*WS*AOS) x denom rank-1 matmuls.  Residual x is added as f16.
"""

import sys

import numpy as np
import ml_dtypes

if "/opt/trn_rl_repo" not in sys.path:
    sys.path.insert(0, "/opt/trn_rl_repo")

import concourse.mybir as mybir
import concourse.tile as tile
from concourse import bacc
from concourse.bass_utils import run_bass_kernel_spmd

F32 = mybir.dt.float32
F32R = mybir.dt.float32r
F16 = mybir.dt.float16
F8 = mybir.dt.float8e4
AF = mybir.ActivationFunctionType
DR = mybir.MatmulPerfMode.DoubleRow
MULT = mybir.AluOpType.mult
ADD = mybir.AluOpType.add
SUB = mybir.AluOpType.subtract

B, N, C = 4, 4096, 512
HALF = N // 2          # own query rows per core
G, GS = 32, 16         # groups, channels per group
P = 128                # partitions
CO = C // P            # channel subtiles (4)
N_CORES = 8
EPS = 1e-6
SM = 1.0 / float(np.sqrt(C))
WS = 32.0              # weight fp8 scale
SHIFT = 2.0            # exp shift (cancels in softmax)
AOS = 1.0 / 64.0       # attn-output fp8 scale
ICH = 512              # query chunk
NCH = HALF // ICH      # 4
JT = N // P            # 32 key tiles
RT = N // 256          # 16 row-pair tiles (stats)
F8NP = ml_dtypes.float8_e4m3
INV_CNT = 1.0 / (N * GS)


def build_nc():
    nc = bacc.Bacc("TRN2", target_bir_lowering=False, num_devices=N_CORES)

    xT8_d = nc.dram_tensor("xT8", [P, CO, N], F8, kind="ExternalInput")
    x8i_d = nc.dram_tensor("x8i", [P, RT, 2 * C], F8, kind="ExternalInput")
    g16_d = nc.dram_tensor("g16", [P, CO, C], F16, kind="ExternalInput")
    h16_d = nc.dram_tensor("h16", [P, CO, C], F16, kind="ExternalInput")
    h8_d = nc.dram_tensor("h8", [P, CO, C], F8, kind="ExternalInput")
    rows_d = nc.dram_tensor("rows", [1, 4 * C], F32, kind="ExternalInput")
    cst_d = nc.dram_tensor("cst", [P, 4], F32R, kind="ExternalInput")
    xr16_d = nc.dram_tensor("xr16", [HALF, C], F16, kind="ExternalInput")
    out_d = nc.dram_tensor("out", [HALF, C], F16, kind="ExternalOutput")

    xr_t = xr16_d[:].rearrange("(t p) c -> t p c", p=P)   # 16 x [128, 512]
    out_t = out_d[:].rearrange("(t p) c -> t p c", p=P)   # 16 x [128, 512]


    with tile.TileContext(nc) as tc:
        with (
            tc.tile_pool(name="persist", bufs=1) as persist,
            tc.tile_pool(name="cpool", bufs=1) as cpool,
        ):
            xT8 = persist.tile([P, CO, N], F8, tag="xT8")
            x8i = persist.tile([P, RT, 2, C], F8, tag="x8i")
            apT8 = persist.tile([P, CO, HALF], F8, tag="apT8")

            cst = cpool.tile([P, 4], F32R, tag="cst")
            ones8 = cpool.tile([P, 2, P], F8, tag="ones8")
            g8 = cpool.tile([P, CO, C], F8, tag="g8")
            h8 = cpool.tile([P, CO, C], F8, tag="h8")
            g16 = cpool.tile([P, CO, C], F16, tag="g16")
            h16 = cpool.tile([P, CO, C], F16, tag="h16")
            irows = cpool.tile([1, 4 * C], F32, tag="irows")
            wrow = cpool.tile([1, 2 * C], F32R, tag="wrow")
            grow = cpool.tile([1, 6 * G], F32, tag="grow")
            junk = cpool.tile([1, 2], F32, tag="junk")
            spart = cpool.tile([P, CO], F32, tag="spart")
            scola = cpool.tile([P, CO], F32, tag="scola")
            scolo = cpool.tile([P, CO], F32, tag="scolo")
            t16p = cpool.tile([P, CO], F16, tag="t16p")
            abias = cpool.tile([P, CO], F32, tag="abias")
            rrow = cpool.tile([1, C], F32R, tag="rrow")
            abrow = cpool.tile([1, C], F32R, tag="abrow")

            gamma_row = irows[:, 0 * C:1 * C]
            beta_row = irows[:, 1 * C:2 * C]
            uq_row = irows[:, 2 * C:3 * C]
            rhost_row = irows[:, 3 * C:4 * C]
            s32_row = wrow[:, 0:C]
            tmp_row = wrow[:, C:2 * C]
            g_Sg = grow[:, 0:G]
            g_Qg = grow[:, G:2 * G]
            g_mean = grow[:, 2 * G:3 * G]
            g_tmp = grow[:, 3 * G:4 * G]
            g_var = grow[:, 4 * G:5 * G]
            g_rstd = grow[:, 5 * G:6 * G]

            # tiny configs first (sync ring), bulk on gpsimd ring
            nc.sync.dma_start(cst[:], cst_d[:])
            nc.sync.dma_start(irows[:], rows_d[:])
            nc.gpsimd.memset(ones8[:], 1.0)

            ones_col = cst[:, 0:1]            # F32R
            ones_2 = cst[:, 0:2]              # F32R [128, 2] of ones
            ones_12r = cst[0:1, 0:2]          # F32R [1, 2] of ones
            ones_11f = cst[0:1, 0:1].bitcast(F32)
            shift_col = cst[:, 2:3].bitcast(F32)
            eps_col = cst[:, 3:4].bitcast(F32)

            # preload the sqrt activation-table set (Square/Copy live in every
            # set, so stats squares and the group-norm Sqrt need no reload)
            nc.scalar.activation(junk[:, 0:1], ones_11f, AF.Sqrt)

            # ---- input DMA schedule ----
            # stats inputs first (they gate everything), xT8 next (first
            # halves prioritized for the A-projection), weights after.
            engs = [nc.gpsimd, nc.scalar, nc.sync]
            for c8 in range(8):
                e = engs[c8 % 3]
                e.dma_start(x8i[:, 2 * c8:2 * c8 + 2]
                            .rearrange("p t two c -> p t (two c)"),
                            x8i_d[:, 2 * c8:2 * c8 + 2, :]
                            .rearrange("p t c -> p t c"))
            nc.gpsimd.dma_start(g16[:], g16_d[:])
            for o in range(CO):       # first halves (A-projection needs these)
                engs[o % 3].dma_start(xT8[:, o, 0:HALF],
                                      xT8_d[:, o, 0:HALF])
            for o in range(CO):
                engs[o % 3].dma_start(xT8[:, o, HALF:N],
                                      xT8_d[:, o, HALF:N])
            nc.scalar.dma_start(h8[:], h8_d[:])
            nc.sync.dma_start(h16[:], h16_d[:])

            # ---- phase 1: group-norm stats ----
            with (
                tc.tile_pool(name="stats_ps", bufs=1, space="PSUM") as stats_ps,
                tc.tile_pool(name="pize_ps", bufs=1, space="PSUM") as pize_ps,
                tc.tile_pool(name="warm_ps", bufs=1, space="PSUM") as warm_ps,
                tc.tile_pool(name="sqpool", bufs=6) as sqpool,
            ):
                def warm(n, tag):
                    w = warm_ps.tile([P, P], F32, tag="warm", name=tag)
                    for wi in range(n):
                        nc.tensor.matmul(w[:], ones8[:], ones8[:],
                                         perf_mode=DR,
                                         start=(wi == 0), stop=(wi == n - 1),
                                         skip_group_check=True)

                warm(24, "w0")
                s_ps = stats_ps.tile([P, C], F32, tag="S")
                q_ps = stats_ps.tile([P, C], F32, tag="Q")
                sq_eng = [nc.scalar, nc.vector, nc.scalar, nc.vector,
                          nc.scalar, nc.vector, nc.scalar, nc.scalar,
                          nc.vector, nc.scalar, nc.vector, nc.scalar,
                          nc.scalar, nc.vector, nc.scalar, nc.scalar]
                sqs = []
                LAG_Q = 4
                for t in range(RT):
                    nc.tensor.matmul(s_ps[:], ones8[:], x8i[:, t],
                                     perf_mode=DR,
                                     start=(t == 0), stop=(t == RT - 1))
                    sq = sqpool.tile([P, 2, C], F8, tag="sq", name=f"sq{t}")
                    sqs.append(sq)
                    e = sq_eng[t]
                    if e is nc.scalar:
                        e.activation(sq[:], x8i[:, t], AF.Square)
                    else:
                        e.tensor_mul(sq[:], x8i[:, t], x8i[:, t])
                    if t >= LAG_Q:
                        nc.tensor.matmul(q_ps[:], ones8[:], sqs[t - LAG_Q],
                                         perf_mode=DR, start=(t == LAG_Q),
                                         stop=False)
                    if t % 4 == 3:
                        warm(4, f"wb{t}")
                for t in range(RT - LAG_Q, RT):
                    nc.tensor.matmul(q_ps[:], ones8[:], sqs[t],
                                     perf_mode=DR, start=False,
                                     stop=(t == RT - 1))
                warm(32, "wbr")   # bridge the group-stats chain (HAM window)

                # group stats chain (vector; Sqrt on scalar, table preloaded)
                nc.vector.reduce_sum(g_Sg,
                                     s_ps[0:1, :].rearrange(
                                         "p (g e) -> p g e", e=GS),
                                     axis=mybir.AxisListType.X)
                nc.vector.tensor_scalar_mul(g_mean, g_Sg, INV_CNT)
                nc.vector.tensor_mul(g_tmp, g_mean, g_mean)
                nc.vector.reduce_sum(g_Qg,
                                     q_ps[0:1, :].rearrange(
                                         "p (g e) -> p g e", e=GS),
                                     axis=mybir.AxisListType.X)
                nc.vector.scalar_tensor_tensor(g_var, g_Qg, INV_CNT, g_tmp,
                                               MULT, SUB)
                nc.scalar.activation(g_rstd, g_var, AF.Sqrt,
                                     bias=eps_col[0:1, :])
                nc.vector.reciprocal(g_rstd, g_rstd)
                # switch scalar table set to exp_and_others off-critical-path
                nc.scalar.activation(junk[:, 1:2], ones_11f, AF.Exp)
                nc.vector.tensor_scalar_mul(g_rstd, g_rstd, WS)  # 32*rstd
                # t path runs on gpsimd in parallel with the s path on vector:
                # tmean = (32*rstd)*mean/32 per group, t = beta - gamma*tmean
                nc.vector.scalar_tensor_tensor(g_tmp, g_rstd, 1.0 / WS,
                                               g_mean, MULT, MULT)
                sv = s32_row.rearrange("p (g e) -> p g e", e=GS)
                tv = tmp_row.rearrange("p (g e) -> p g e", e=GS)
                gv = gamma_row.rearrange("p (g e) -> p g e", e=GS)
                nc.vector.tensor_tensor(
                    sv, gv, g_rstd[:, :, None].to_broadcast((1, G, GS)), MULT)
                nc.vector.tensor_tensor(
                    tv, gv, g_tmp[:, :, None].to_broadcast((1, G, GS)), MULT)
                nc.vector.tensor_sub(tmp_row, beta_row.bitcast(F32R), tmp_row)

                # partition-ize 32*s  ([1,512] row -> [128,4]) on the PE (the
                # pize matmuls are the head of the phase-2 queue; f32r avoids
                # the fp32 two-pass matmul split)
                pp = pize_ps.tile([P, CO, 2], F32, tag="pize", name="pp")
                for o in range(CO):
                    nc.tensor.matmul(pp[:, o, :],
                                     s32_row[0:1, o * P:(o + 1) * P],
                                     ones_12r,
                                     start=(o == 0), stop=(o == CO - 1))
                nc.scalar.activation(spart[:], pp[:, :, 0], AF.Copy)
                nc.vector.tensor_scalar_mul(scola[:], spart[:],
                                            1.0 / (WS * WS))
                nc.vector.tensor_scalar_mul(scolo[:], spart[:], AOS / WS)

                # fold 32*s into fp8 G (scalar + vector; gpsimd is slow on
                # f16 and its SBUF traffic stalls the DVE port)
                for ci in range(CO):
                    if ci % 2 == 0:
                        nc.scalar.activation(g8[:, ci, :], g16[:, ci, :],
                                             AF.Copy,
                                             scale=spart[:, ci:ci + 1])
                    else:
                        nc.vector.tensor_scalar_mul(
                            g8[:, ci, :], g16[:, ci, :],
                            spart[:, ci:ci + 1])

            # ---- phase 2: A-projection + bias rows ----
            cpc = [0]

            def a_copy(ps, o, win):
                e = cpc[0] % 2
                cpc[0] += 1
                dst = apT8[:, o, win * ICH:(win + 1) * ICH]
                if e == 0:
                    nc.scalar.activation(dst, ps[:], AF.Identity,
                                         bias=abias[:, o:o + 1],
                                         scale=scola[:, o:o + 1])
                else:
                    nc.vector.tensor_scalar(dst, ps[:],
                                            scola[:, o:o + 1],
                                            abias[:, o:o + 1], MULT, ADD)

            with (
                tc.tile_pool(name="proj_ps", bufs=5, space="PSUM") as proj_ps,
                tc.tile_pool(name="aux_ps", bufs=1, space="PSUM") as aux_ps,
            ):
                def a_mms(o, win):
                    ps = proj_ps.tile([P, ICH], F32, tag="proj",
                                      name=f"a{o}_{win}")
                    for u in range(2):
                        nc.tensor.matmul(
                            ps[:],
                            g8[:, 2 * u:2 * u + 2, o * P:(o + 1) * P],
                            xT8[:, 2 * u:2 * u + 2,
                                win * ICH:(win + 1) * ICH],
                            perf_mode=DR, start=(u == 0), stop=(u == 1))
                    return ps

                def a_group(o, win):
                    a_copy(a_mms(o, win), o, win)

                # win-major so chunk-0 A columns complete first; win-0 copies
                # are deferred until abias exists (read-before-write hazard)
                ps0 = [a_mms(o, 0) for o in range(CO)]
                # partition-ize t ([1,512] -> [128,4] f16) via tiny matmuls
                pp = aux_ps.tile([P, CO, 2], F32, tag="pize", name="ppt")
                for o in range(CO):
                    nc.tensor.matmul(pp[:, o, :],
                                     tmp_row[0:1, o * P:(o + 1) * P],
                                     ones_12r,
                                     start=(o == 0), stop=(o == CO - 1))
                nc.vector.tensor_copy(t16p[:], pp[:, :, 0])
                # teff = t @ G  (f16), then abias = s*(teff+uq)/32
                bps = aux_ps.tile([1, C], F32, tag="bps")
                for o in range(CO):
                    nc.tensor.matmul(bps[:], t16p[:, o:o + 1], g16[:, o, :],
                                     start=(o == 0), stop=(o == CO - 1))
                nc.vector.tensor_add(abrow[:], bps[:], uq_row)
                nc.vector.scalar_tensor_tensor(abrow[:], abrow[:], 1.0 / WS,
                                               s32_row.bitcast(F32), MULT,
                                               MULT)
                ppa = aux_ps.tile([P, CO, 2], F32, tag="pize", name="ppa")
                for o in range(CO):
                    nc.tensor.matmul(ppa[:, o, :],
                                     abrow[0:1, o * P:(o + 1) * P],
                                     ones_12r,
                                     start=(o == 0), stop=(o == CO - 1))
                nc.vector.tensor_copy(abias[:], ppa[:, :, 0])
                for o in range(CO):
                    a_copy(ps0[o], o, 0)
                for o in range(CO):
                    a_group(o, 1)
                # tH row -> R_used = (tH + bv@wo + bo)*WS*AOS
                ths = aux_ps.tile([1, C], F32, tag="ths")
                for o in range(CO):
                    nc.tensor.matmul(ths[:], t16p[:, o:o + 1], h16[:, o, :],
                                     start=(o == 0), stop=(o == CO - 1))
                nc.vector.scalar_tensor_tensor(rrow[:], ths[:],
                                               WS * AOS, rhost_row, MULT, ADD)
                for win in range(2, NCH):
                    for o in range(CO):
                        a_group(o, win)

            # ---- phase 3: attention + O-projection + residual ----
            with (
                tc.tile_pool(name="av_ps", bufs=1, space="PSUM") as av_ps,
                tc.tile_pool(name="sps_ps", bufs=3, space="PSUM") as sps_ps,
                tc.tile_pool(name="op_ps", bufs=1, space="PSUM") as op_ps,
                tc.tile_pool(name="expp", bufs=5) as expp,
                tc.tile_pool(name="accp", bufs=2) as accp,
                tc.tile_pool(name="aoTp", bufs=2) as aoTp,
                tc.tile_pool(name="drow", bufs=2) as drow,
                tc.tile_pool(name="xres", bufs=6) as xres,
                tc.tile_pool(name="ostage", bufs=2) as ostage,
            ):
                LAG = 3  # AV pairs trail scores by 3 so tail MMs interleave

                def make_tail(ch, avs, acc_a, acc_b, last=False):
                    """Chunk-end work, split into pieces emitted between the
                    next chunk's score matmuls."""
                    st = {}

                    pool, ptag = (sps_ps, "sps") if last else (op_ps, "op")
                    if last:
                        xrs = []
                        for it in range(CO):
                            xr = xres.tile([P, C], F16, tag="xr",
                                           name=f"xrL{it}")
                            nc.sync.dma_start(xr[:], xr_t[ch * CO + it])
                            xrs.append(xr)

                    def p0():
                        nc.vector.tensor_add(acc_a[:], acc_a[:], acc_b[:])
                        dps = pool.tile([1, ICH], F32, tag=ptag,
                                        name=f"dps{ch}")
                        nc.tensor.matmul(dps[:], ones_col.bitcast(F32),
                                         acc_a[:], start=True, stop=True)
                        d_row = drow.tile([1, ICH], F32R, tag="d_row",
                                          name=f"drow{ch}")
                        nc.vector.tensor_copy(d_row[:], dps[:])
                        st["d_row"] = d_row

                    def p2():
                        d_row = st["d_row"]
                        dp = pool.tile([P, CO, 2], F32, tag=ptag,
                                       name=f"dp{ch}")
                        for o in range(CO):
                            nc.tensor.matmul(dp[:, o, :],
                                             d_row[0:1, o * P:(o + 1) * P],
                                             ones_12r,
                                             start=(o == 0),
                                             stop=(o == CO - 1))
                        d_inv = drow.tile([P, CO], F32, tag="d_inv",
                                          name=f"dinv{ch}")
                        nc.vector.tensor_scalar_mul(d_inv[:], dp[:, :, 0],
                                                    WS * AOS)
                        nc.vector.reciprocal(d_inv[:], d_inv[:])
                        aoT = aoTp.tile([P, CO, ICH], F8, tag="aoT",
                                        name=f"aoT{ch}")
                        for cs in range(CO):
                            if cs % 2 == 0:
                                nc.vector.tensor_scalar_mul(
                                    aoT[:, cs, :], avs[cs][:],
                                    scolo[:, cs:cs + 1])
                            else:
                                nc.scalar.activation(
                                    aoT[:, cs, :], avs[cs][:], AF.Copy,
                                    scale=scolo[:, cs:cs + 1])
                        st["d_inv"] = d_inv
                        st["aoT"] = aoT

                    def mk_it(it):
                        def p():
                            aoT, d_inv = st["aoT"], st["d_inv"]
                            d_row = st["d_row"]
                            ops = pool.tile([P, C], F32, tag=ptag,
                                            name=f"o{ch}_{it}")
                            for u in range(2):
                                nc.tensor.matmul(
                                    ops[:],
                                    aoT[:, 2 * u:2 * u + 2,
                                        it * P:(it + 1) * P],
                                    h8[:, 2 * u:2 * u + 2, :],
                                    perf_mode=DR, start=(u == 0),
                                    stop=False)
                            nc.tensor.matmul(
                                ops[:],
                                d_row[0:1, it * P:(it + 1) * P],
                                rrow[:], start=False, stop=True)
                            if last:
                                xr = xrs[it]
                            else:
                                xr = xres.tile([P, C], F16, tag="xr",
                                               name=f"xr{ch}_{it}")
                                nc.sync.dma_start(xr[:], xr_t[ch * CO + it])
                            ot = ostage.tile([P, C], F16, tag="ot",
                                             name=f"ot{ch}_{it}")
                            nc.vector.scalar_tensor_tensor(
                                ot[:], ops[:], d_inv[:, it:it + 1], xr[:],
                                MULT, ADD)
                            nc.sync.dma_start(out_t[ch * CO + it], ot[:])
                        return p

                    def noop():
                        pass

                    return [p0, noop, p2, mk_it(0), mk_it(1), mk_it(2),
                            mk_it(3)]

                tail = []
                for ch in range(NCH):
                    i0 = ch * ICH
                    avs = [av_ps.tile([P, ICH], F32, tag=f"av{i}",
                                      name=f"av{ch}_{i}")
                           for i in range(CO)]
                    acc_a = accp.tile([P, ICH], F32, tag="acc_a",
                                      name=f"acca{ch}")
                    acc_b = accp.tile([P, ICH], F32, tag="acc_b",
                                      name=f"accb{ch}")

                    def scores(j, ex, jj, i0=i0, acc_a=acc_a, acc_b=acc_b,
                               ch=ch):
                        sps = sps_ps.tile([P, ICH], F32, tag="sps",
                                          name=f"sps{ch}_{j}")
                        for u in range(2):
                            nc.tensor.matmul(
                                sps[:],
                                xT8[:, 2 * u:2 * u + 2, j * P:(j + 1) * P],
                                apT8[:, 2 * u:2 * u + 2, i0:i0 + ICH],
                                perf_mode=DR, start=(u == 0), stop=(u == 1))
                        nc.scalar.activation(ex[:, jj, :], sps[:], AF.Exp,
                                             bias=shift_col, scale=SM)
                        if jj == 0:
                            if j == 0:
                                nc.vector.tensor_copy(acc_a[:], ex[:, 0, :])
                            else:
                                nc.vector.tensor_add(acc_a[:], acc_a[:],
                                                     ex[:, 0, :])
                        else:
                            if j == 1:
                                nc.gpsimd.tensor_copy(acc_b[:], ex[:, 1, :])
                            else:
                                nc.gpsimd.tensor_add(acc_b[:], acc_b[:],
                                                     ex[:, 1, :])

                    def av_mms(t, ex, avs=avs):
                        for cs in range(CO):
                            nc.tensor.matmul(
                                avs[cs][:],
                                x8i[:, t, :, cs * P:(cs + 1) * P],
                                ex[:],
                                perf_mode=DR, start=(t == 0),
                                stop=(t == JT // 2 - 1))

                    lag = 1 if ch == NCH - 1 else LAG
                    exs = {}
                    for t in range(JT // 2):
                        ex = expp.tile([P, 2, ICH], F8, tag="ex",
                                       name=f"ex{ch}_{t}")
                        exs[t] = ex
                        scores(2 * t, ex, 0)
                        scores(2 * t + 1, ex, 1)
                        if 2 <= t <= len(tail) + 1:
                            tail[t - 2]()
                        if t >= lag:
                            av_mms(t - lag, exs.pop(t - lag))
                    for t in range(JT // 2 - lag, JT // 2):
                        av_mms(t, exs.pop(t))
                    tail = make_tail(ch, avs, acc_a, acc_b,
                                     last=(ch == NCH - 1))

                def warm_tail(n, tag):
                    w = sps_ps.tile([P, P], F32, tag="sps", name=tag)
                    for wi in range(n):
                        nc.tensor.matmul(w[:], ones8[:], ones8[:],
                                         perf_mode=DR,
                                         start=(wi == 0), stop=(wi == n - 1),
                                         skip_group_check=True)

                for i, piece in enumerate(tail):
                    piece()
                    if i in (0, 2):
                        warm_tail(8, f"wt{i}")

    nc.compile()
    return nc


_NC = None


def _get_nc():
    global _NC
    if _NC is None:
        _NC = build_nc()
    return _NC


def make_in_maps(x, gn_gamma, gn_beta, wq, bq, wk, bk, wv, bv, wo, bo):
    x4 = np.asarray(x, np.float32).reshape(B, N, C)
    wq, bq = np.asarray(wq, np.float32), np.asarray(bq, np.float32)
    wk, bk = np.asarray(wk, np.float32), np.asarray(bk, np.float32)
    wv, bv = np.asarray(wv, np.float32), np.asarray(bv, np.float32)
    wo, bo = np.asarray(wo, np.float32), np.asarray(bo, np.float32)

    def wlay(w):
        return np.asarray(w, np.float32).reshape(CO, P, C).transpose(1, 0, 2)

    Gm = wq @ wk.T
    Hm = wv @ wo
    uq = bq @ wk.T
    rhost = (bv @ wo + bo) * (WS * AOS)

    rows = np.zeros((1, 4 * C), np.float32)
    for i, v in enumerate((gn_gamma, gn_beta)):
        rows[0, i * C:(i + 1) * C] = np.asarray(v, np.float32)
    rows[0, 2 * C:3 * C] = uq
    rows[0, 3 * C:4 * C] = rhost
    cst = np.zeros((P, 4), np.float32)
    cst[:, 0] = 1.0
    cst[:, 1] = 1.0
    cst[:, 2] = -SHIFT
    cst[:, 3] = EPS
    common = dict(
        g16=np.ascontiguousarray(wlay(Gm).astype(np.float16)),
        h16=np.ascontiguousarray(wlay(Hm).astype(np.float16)),
        h8=np.ascontiguousarray((WS * wlay(Hm)).astype(F8NP)),
        rows=rows, cst=cst,
    )
    in_maps = []
    for c in range(N_CORES):
        b, h = c // 2, c % 2
        own = x4[b, h * HALF:(h + 1) * HALF]
        other = x4[b, (1 - h) * HALF:(2 - h) * HALF]
        xp = np.concatenate([own, other], axis=0)        # [N, C]
        xp8 = xp.astype(F8NP)
        xT8 = np.ascontiguousarray(                      # [P, CO, N] p-major
            xp8.T.reshape(CO, P, N).transpose(1, 0, 2))
        x8i = np.ascontiguousarray(                      # [P, RT, 2C] p-major
            xp8.reshape(RT, 2, P, C).transpose(2, 0, 1, 3)
               .reshape(P, RT, 2 * C))
        xr16 = np.ascontiguousarray(own.astype(np.float16))
        in_maps.append(dict(xT8=xT8, x8i=x8i, xr16=xr16, **common))
    return in_maps


def assemble(results):
    out = np.empty((B, N, C), np.float32)
    for c in range(N_CORES):
        b, h = c // 2, c % 2
        out[b, h * HALF:(h + 1) * HALF] = results[c]["out"].astype(np.float32)
    return out.reshape(B, 64, 64, C)


def kernel(**inputs):
    nc = _get_nc()
    in_maps = make_in_maps(**inputs)
    res = run_bass_kernel_spmd(nc, in_maps, list(range(N_CORES)))
    return assemble(res.results)


# revision 13
# speedup vs baseline: 1.1387x; 1.0321x over previous
"""Trainium2 Bass kernel for a spatial self-attention block (fp8 DoubleRow).

reference computation (B=4, H=W=64, C=512, N=H*W=4096):
    h = group_norm(x, gamma, beta, 32 groups)
    q,k,v = h@wq+bq, h@wk+bk, h@wv+bv
    scores = (q @ k^T) / sqrt(C); attn = softmax(scores, -1)
    out = (attn @ v) @ wo + bo + x

Folded-weight form (eliminates the K and V projections entirely):
    G = wq @ wk^T, H = wv @ wo   (host-precomputed)
    scores[i,j] = (s*(h_i G) + s*(bq wk^T)) . x_j   + const_i  (cancels in softmax)
    out_i = (s*(exps_i @ x)) H / denom_i + tH + bv wo + bo + x_i
so the device only computes: group-norm stats -> A-projection (A' = s*(hG)+...)
-> scores = A' . x^T -> AV = exps @ x -> O-projection via H.  The row terms
(tG, tH) are rank-1 corrections computed with tiny matmuls; per-row constants
drop out of softmax.

Sharding: 8 cores = (batch b in 0..3) x (query-half in 0..1); each core
computes stats over its full batch element and attention for its 2048 rows.

All heavy matmuls are fp8(e4m3) DoubleRow ([128, 2, free] APs, contraction
over partition x pair = 256 per pass).  Scaling scheme:
  - G is folded with diag(32*s) input-side into fp8 (WS=32); A' output copy
    applies s/(32*WS) as a per-partition scale column plus the bias column
    s*(tG + bq wk^T)/32.
  - exp uses a fixed shift (cancels in softmax); ex stored fp8.
  - AV accumulates exps @ raw-x; the O-copy scale column is s*AOS/32; H is
    host-scaled by WS into fp8.  After the O-projection the result is
    multiplied by 1/(WS*AOS*denom) and the row R = tH + bv wo + bo enters as
    (R*WS*AOS) x denom rank-1 matmuls.  Residual x is added as f16.
"""

import sys

import numpy as np
import ml_dtypes

if "/opt/trn_rl_repo" not in sys.path:
    sys.path.insert(0, "/opt/trn_rl_repo")

import concourse.mybir as mybir
import concourse.tile as tile
from concourse import bacc
from concourse.bass_utils import run_bass_kernel_spmd

F32 = mybir.dt.float32
F32R = mybir.dt.float32r
F16 = mybir.dt.float16
F8 = mybir.dt.float8e4
AF = mybir.ActivationFunctionType
DR = mybir.MatmulPerfMode.DoubleRow
MULT = mybir.AluOpType.mult
ADD = mybir.AluOpType.add
SUB = mybir.AluOpType.subtract

B, N, C = 4, 4096, 512
HALF = N // 2          # own query rows per core
G, GS = 32, 16         # groups, channels per group
P = 128                # partitions
CO = C // P            # channel subtiles (4)
N_CORES = 8
EPS = 1e-6
SM = 1.0 / float(np.sqrt(C))
WS = 32.0              # weight fp8 scale
SHIFT = 2.0            # exp shift (cancels in softmax)
AOS = 1.0 / 64.0       # attn-output fp8 scale
ICH = 512              # query chunk
NCH = HALF // ICH      # 4
JT = N // P            # 32 key tiles
RT = N // 256          # 16 row-pair tiles (stats)
F8NP = ml_dtypes.float8_e4m3
INV_CNT = 1.0 / (N * GS)


def build_nc():
    nc = bacc.Bacc("TRN2", target_bir_lowering=False, num_devices=N_CORES)

    xT8_d = nc.dram_tensor("xT8", [P, CO, N], F8, kind="ExternalInput")
    x8i_d = nc.dram_tensor("x8i", [P, RT, 2 * C], F8, kind="ExternalInput")
    g16_d = nc.dram_tensor("g16", [P, CO, C], F16, kind="ExternalInput")
    h16_d = nc.dram_tensor("h16", [P, CO, C], F16, kind="ExternalInput")
    h8_d = nc.dram_tensor("h8", [P, CO, C], F8, kind="ExternalInput")
    rows_d = nc.dram_tensor("rows", [1, 4 * C], F32, kind="ExternalInput")
    cst_d = nc.dram_tensor("cst", [P, 4], F32R, kind="ExternalInput")
    xr16_d = nc.dram_tensor("xr16", [HALF, C], F16, kind="ExternalInput")
    out_d = nc.dram_tensor("out", [HALF, C], F16, kind="ExternalOutput")

    xr_t = xr16_d[:].rearrange("(t p) c -> t p c", p=P)   # 16 x [128, 512]
    out_t = out_d[:].rearrange("(t p) c -> t p c", p=P)   # 16 x [128, 512]


    with tile.TileContext(nc) as tc:
        with (
            tc.tile_pool(name="persist", bufs=1) as persist,
            tc.tile_pool(name="cpool", bufs=1) as cpool,
        ):
            xT8 = persist.tile([P, CO, N], F8, tag="xT8")
            x8i = persist.tile([P, RT, 2, C], F8, tag="x8i")
            apT8 = persist.tile([P, CO, HALF], F8, tag="apT8")

            cst = cpool.tile([P, 4], F32R, tag="cst")
            ones8 = cpool.tile([P, 2, P], F8, tag="ones8")
            g8 = cpool.tile([P, CO, C], F8, tag="g8")
            h8 = cpool.tile([P, CO, C], F8, tag="h8")
            g16 = cpool.tile([P, CO, C], F16, tag="g16")
            h16 = cpool.tile([P, CO, C], F16, tag="h16")
            irows = cpool.tile([1, 4 * C], F32, tag="irows")
            wrow = cpool.tile([1, 2 * C], F32R, tag="wrow")
            grow = cpool.tile([1, 6 * G], F32, tag="grow")
            junk = cpool.tile([1, 2], F32, tag="junk")
            spart = cpool.tile([P, CO], F32, tag="spart")
            scola = cpool.tile([P, CO], F32, tag="scola")
            scolo = cpool.tile([P, CO], F32, tag="scolo")
            t16p = cpool.tile([P, CO], F16, tag="t16p")
            abias = cpool.tile([P, CO], F32, tag="abias")
            rrow = cpool.tile([1, C], F32R, tag="rrow")
            abrow = cpool.tile([1, C], F32R, tag="abrow")

            gamma_row = irows[:, 0 * C:1 * C]
            beta_row = irows[:, 1 * C:2 * C]
            uq_row = irows[:, 2 * C:3 * C]
            rhost_row = irows[:, 3 * C:4 * C]
            s32_row = wrow[:, 0:C]
            tmp_row = wrow[:, C:2 * C]
            g_Sg = grow[:, 0:G]
            g_Qg = grow[:, G:2 * G]
            g_mean = grow[:, 2 * G:3 * G]
            g_tmp = grow[:, 3 * G:4 * G]
            g_var = grow[:, 4 * G:5 * G]
            g_rstd = grow[:, 5 * G:6 * G]

            # tiny configs first (sync ring), bulk on gpsimd ring
            nc.sync.dma_start(cst[:], cst_d[:])
            nc.sync.dma_start(irows[:], rows_d[:])
            nc.gpsimd.memset(ones8[:], 1.0)

            ones_col = cst[:, 0:1]            # F32R
            ones_2 = cst[:, 0:2]              # F32R [128, 2] of ones
            ones_12r = cst[0:1, 0:2]          # F32R [1, 2] of ones
            ones_11f = cst[0:1, 0:1].bitcast(F32)
            shift_col = cst[:, 2:3].bitcast(F32)
            eps_col = cst[:, 3:4].bitcast(F32)

            # preload the sqrt activation-table set (Square/Copy live in every
            # set, so stats squares and the group-norm Sqrt need no reload)
            nc.scalar.activation(junk[:, 0:1], ones_11f, AF.Sqrt)

            # ---- input DMA schedule ----
            # stats inputs first (they gate everything), xT8 next (first
            # halves prioritized for the A-projection), weights after.
            # all bulk inputs on ONE ring (gpsimd): ring FIFO guarantees the
            # stats stream (x8i) gets the full fabric before xT8/weights
            for c8 in range(8):
                nc.gpsimd.dma_start(x8i[:, 2 * c8:2 * c8 + 2]
                                    .rearrange("p t two c -> p t (two c)"),
                                    x8i_d[:, 2 * c8:2 * c8 + 2, :]
                                    .rearrange("p t c -> p t c"))
            nc.gpsimd.dma_start(g16[:], g16_d[:])
            for o in range(CO):       # first halves (A-projection needs these)
                nc.gpsimd.dma_start(xT8[:, o, 0:HALF], xT8_d[:, o, 0:HALF])
            for o in range(CO):
                nc.gpsimd.dma_start(xT8[:, o, HALF:N], xT8_d[:, o, HALF:N])
            nc.gpsimd.dma_start(h8[:], h8_d[:])
            nc.gpsimd.dma_start(h16[:], h16_d[:])

            # ---- phase 1: group-norm stats ----
            with (
                tc.tile_pool(name="stats_ps", bufs=1, space="PSUM") as stats_ps,
                tc.tile_pool(name="pize_ps", bufs=1, space="PSUM") as pize_ps,
                tc.tile_pool(name="warm_ps", bufs=1, space="PSUM") as warm_ps,
                tc.tile_pool(name="sqpool", bufs=6) as sqpool,
            ):
                def warm(n, tag):
                    w = warm_ps.tile([P, P], F32, tag="warm", name=tag)
                    for wi in range(n):
                        nc.tensor.matmul(w[:], ones8[:], ones8[:],
                                         perf_mode=DR,
                                         start=(wi == 0), stop=(wi == n - 1),
                                         skip_group_check=True)

                warm(24, "w0")
                s_ps = stats_ps.tile([P, C], F32, tag="S")
                q_ps = stats_ps.tile([P, C], F32, tag="Q")
                sq_eng = [nc.scalar, nc.vector, nc.scalar, nc.vector,
                          nc.scalar, nc.vector, nc.scalar, nc.scalar,
                          nc.vector, nc.scalar, nc.vector, nc.scalar,
                          nc.scalar, nc.vector, nc.scalar, nc.scalar]
                sqs = []
                LAG_Q = 4
                for t in range(RT):
                    nc.tensor.matmul(s_ps[:], ones8[:], x8i[:, t],
                                     perf_mode=DR,
                                     start=(t == 0), stop=(t == RT - 1))
                    sq = sqpool.tile([P, 2, C], F8, tag="sq", name=f"sq{t}")
                    sqs.append(sq)
                    e = sq_eng[t]
                    if e is nc.scalar:
                        e.activation(sq[:], x8i[:, t], AF.Square)
                    else:
                        e.tensor_mul(sq[:], x8i[:, t], x8i[:, t])
                    if t >= LAG_Q:
                        nc.tensor.matmul(q_ps[:], ones8[:], sqs[t - LAG_Q],
                                         perf_mode=DR, start=(t == LAG_Q),
                                         stop=False)
                    if t % 4 == 3:
                        warm(4, f"wb{t}")
                for t in range(RT - LAG_Q, RT):
                    nc.tensor.matmul(q_ps[:], ones8[:], sqs[t],
                                     perf_mode=DR, start=False,
                                     stop=(t == RT - 1))
                warm(32, "wbr")   # bridge the group-stats chain (HAM window)

                # group stats chain (vector; Sqrt on scalar, table preloaded)
                nc.vector.reduce_sum(g_Sg,
                                     s_ps[0:1, :].rearrange(
                                         "p (g e) -> p g e", e=GS),
                                     axis=mybir.AxisListType.X)
                nc.vector.tensor_scalar_mul(g_mean, g_Sg, INV_CNT)
                nc.vector.tensor_mul(g_tmp, g_mean, g_mean)
                nc.vector.reduce_sum(g_Qg,
                                     q_ps[0:1, :].rearrange(
                                         "p (g e) -> p g e", e=GS),
                                     axis=mybir.AxisListType.X)
                nc.vector.scalar_tensor_tensor(g_var, g_Qg, INV_CNT, g_tmp,
                                               MULT, SUB)
                nc.scalar.activation(g_rstd, g_var, AF.Sqrt,
                                     bias=eps_col[0:1, :])
                nc.vector.reciprocal(g_rstd, g_rstd)
                # switch scalar table set to exp_and_others off-critical-path
                # (reads g_rstd so the scheduler cannot hoist it before Sqrt)
                nc.scalar.activation(junk[:, 1:2], g_rstd[0:1, 0:1], AF.Exp)
                nc.vector.tensor_scalar_mul(g_rstd, g_rstd, WS)  # 32*rstd
                # t path runs on gpsimd in parallel with the s path on vector:
                # tmean = (32*rstd)*mean/32 per group, t = beta - gamma*tmean
                nc.vector.scalar_tensor_tensor(g_tmp, g_rstd, 1.0 / WS,
                                               g_mean, MULT, MULT)
                sv = s32_row.rearrange("p (g e) -> p g e", e=GS)
                tv = tmp_row.rearrange("p (g e) -> p g e", e=GS)
                gv = gamma_row.rearrange("p (g e) -> p g e", e=GS)
                nc.vector.tensor_tensor(
                    sv, gv, g_rstd[:, :, None].to_broadcast((1, G, GS)), MULT)
                nc.vector.tensor_tensor(
                    tv, gv, g_tmp[:, :, None].to_broadcast((1, G, GS)), MULT)
                nc.vector.tensor_sub(tmp_row, beta_row.bitcast(F32R), tmp_row)

                # partition-ize 32*s  ([1,512] row -> [128,4]) on the PE (the
                # pize matmuls are the head of the phase-2 queue; f32r avoids
                # the fp32 two-pass matmul split)
                pp = pize_ps.tile([P, CO, 2], F32, tag="pize", name="pp")
                for o in range(CO):
                    nc.tensor.matmul(pp[:, o, :],
                                     s32_row[0:1, o * P:(o + 1) * P],
                                     ones_12r,
                                     start=(o == 0), stop=(o == CO - 1))
                nc.scalar.activation(spart[:], pp[:, :, 0], AF.Copy)
                nc.vector.tensor_scalar_mul(scola[:], spart[:],
                                            1.0 / (WS * WS))
                nc.vector.tensor_scalar_mul(scolo[:], spart[:], AOS / WS)

                # fold 32*s into fp8 G (scalar + vector; gpsimd is slow on
                # f16 and its SBUF traffic stalls the DVE port)
                for ci in range(CO):
                    if ci % 2 == 0:
                        nc.scalar.activation(g8[:, ci, :], g16[:, ci, :],
                                             AF.Copy,
                                             scale=spart[:, ci:ci + 1])
                    else:
                        nc.vector.tensor_scalar_mul(
                            g8[:, ci, :], g16[:, ci, :],
                            spart[:, ci:ci + 1])

            # ---- phase 2: A-projection + bias rows ----
            cpc = [0]

            def a_copy(ps, o, win):
                e = cpc[0] % 2
                cpc[0] += 1
                dst = apT8[:, o, win * ICH:(win + 1) * ICH]
                if e == 0:
                    nc.scalar.activation(dst, ps[:], AF.Identity,
                                         bias=abias[:, o:o + 1],
                                         scale=scola[:, o:o + 1])
                else:
                    nc.vector.tensor_scalar(dst, ps[:],
                                            scola[:, o:o + 1],
                                            abias[:, o:o + 1], MULT, ADD)

            with (
                tc.tile_pool(name="proj_ps", bufs=5, space="PSUM") as proj_ps,
                tc.tile_pool(name="aux_ps", bufs=1, space="PSUM") as aux_ps,
            ):
                def a_mms(o, win):
                    ps = proj_ps.tile([P, ICH], F32, tag="proj",
                                      name=f"a{o}_{win}")
                    for u in range(2):
                        nc.tensor.matmul(
                            ps[:],
                            g8[:, 2 * u:2 * u + 2, o * P:(o + 1) * P],
                            xT8[:, 2 * u:2 * u + 2,
                                win * ICH:(win + 1) * ICH],
                            perf_mode=DR, start=(u == 0), stop=(u == 1))
                    return ps

                def a_group(o, win):
                    a_copy(a_mms(o, win), o, win)

                # win-major so chunk-0 A columns complete first; win-0 copies
                # are deferred until abias exists (read-before-write hazard)
                ps0 = [a_mms(o, 0) for o in range(CO)]
                # partition-ize t ([1,512] -> [128,4] f16) via tiny matmuls
                pp = aux_ps.tile([P, CO, 2], F32, tag="pize", name="ppt")
                for o in range(CO):
                    nc.tensor.matmul(pp[:, o, :],
                                     tmp_row[0:1, o * P:(o + 1) * P],
                                     ones_12r,
                                     start=(o == 0), stop=(o == CO - 1))
                nc.vector.tensor_copy(t16p[:], pp[:, :, 0])
                # teff = t @ G  (f16), then abias = s*(teff+uq)/32
                bps = aux_ps.tile([1, C], F32, tag="bps")
                for o in range(CO):
                    nc.tensor.matmul(bps[:], t16p[:, o:o + 1], g16[:, o, :],
                                     start=(o == 0), stop=(o == CO - 1))
                nc.vector.tensor_add(abrow[:], bps[:], uq_row)
                nc.vector.scalar_tensor_tensor(abrow[:], abrow[:], 1.0 / WS,
                                               s32_row.bitcast(F32), MULT,
                                               MULT)
                ppa = aux_ps.tile([P, CO, 2], F32, tag="pize", name="ppa")
                for o in range(CO):
                    nc.tensor.matmul(ppa[:, o, :],
                                     abrow[0:1, o * P:(o + 1) * P],
                                     ones_12r,
                                     start=(o == 0), stop=(o == CO - 1))
                nc.vector.tensor_copy(abias[:], ppa[:, :, 0])
                for o in range(CO):
                    a_copy(ps0[o], o, 0)
                for o in range(CO):
                    a_group(o, 1)
                # tH row -> R_used = (tH + bv@wo + bo)*WS*AOS
                ths = aux_ps.tile([1, C], F32, tag="ths")
                for o in range(CO):
                    nc.tensor.matmul(ths[:], t16p[:, o:o + 1], h16[:, o, :],
                                     start=(o == 0), stop=(o == CO - 1))
                nc.vector.scalar_tensor_tensor(rrow[:], ths[:],
                                               WS * AOS, rhost_row, MULT, ADD)
                for win in range(2, NCH):
                    for o in range(CO):
                        a_group(o, win)

            # ---- phase 3: attention + O-projection + residual ----
            with (
                tc.tile_pool(name="av_ps", bufs=1, space="PSUM") as av_ps,
                tc.tile_pool(name="sps_ps", bufs=3, space="PSUM") as sps_ps,
                tc.tile_pool(name="op_ps", bufs=1, space="PSUM") as op_ps,
                tc.tile_pool(name="expp", bufs=5) as expp,
                tc.tile_pool(name="accp", bufs=2) as accp,
                tc.tile_pool(name="aoTp", bufs=2) as aoTp,
                tc.tile_pool(name="drow", bufs=2) as drow,
                tc.tile_pool(name="xres", bufs=6) as xres,
                tc.tile_pool(name="ostage", bufs=2) as ostage,
            ):
                LAG = 3  # AV pairs trail scores by 3 so tail MMs interleave

                def make_tail(ch, avs, acc_a, acc_b, last=False):
                    """Chunk-end work, split into pieces emitted between the
                    next chunk's score matmuls."""
                    st = {}

                    pool, ptag = (sps_ps, "sps") if last else (op_ps, "op")
                    if last:
                        xrs = []
                        for it in range(CO):
                            xr = xres.tile([P, C], F16, tag="xr",
                                           name=f"xrL{it}")
                            nc.sync.dma_start(xr[:], xr_t[ch * CO + it])
                            xrs.append(xr)

                    def p0():
                        nc.vector.tensor_add(acc_a[:], acc_a[:], acc_b[:])
                        dps = pool.tile([1, ICH], F32, tag=ptag,
                                        name=f"dps{ch}")
                        nc.tensor.matmul(dps[:], ones_col.bitcast(F32),
                                         acc_a[:], start=True, stop=True)
                        d_row = drow.tile([1, ICH], F32R, tag="d_row",
                                          name=f"drow{ch}")
                        nc.vector.tensor_copy(d_row[:], dps[:])
                        st["d_row"] = d_row

                    def p2():
                        d_row = st["d_row"]
                        dp = pool.tile([P, CO, 2], F32, tag=ptag,
                                       name=f"dp{ch}")
                        for o in range(CO):
                            nc.tensor.matmul(dp[:, o, :],
                                             d_row[0:1, o * P:(o + 1) * P],
                                             ones_12r,
                                             start=(o == 0),
                                             stop=(o == CO - 1))
                        d_inv = drow.tile([P, CO], F32, tag="d_inv",
                                          name=f"dinv{ch}")
                        nc.vector.tensor_scalar_mul(d_inv[:], dp[:, :, 0],
                                                    WS * AOS)
                        nc.vector.reciprocal(d_inv[:], d_inv[:])
                        aoT = aoTp.tile([P, CO, ICH], F8, tag="aoT",
                                        name=f"aoT{ch}")
                        for cs in range(CO):
                            if cs % 2 == 0:
                                nc.vector.tensor_scalar_mul(
                                    aoT[:, cs, :], avs[cs][:],
                                    scolo[:, cs:cs + 1])
                            else:
                                nc.scalar.activation(
                                    aoT[:, cs, :], avs[cs][:], AF.Copy,
                                    scale=scolo[:, cs:cs + 1])
                        st["d_inv"] = d_inv
                        st["aoT"] = aoT

                    def mk_it(it):
                        def p():
                            aoT, d_inv = st["aoT"], st["d_inv"]
                            d_row = st["d_row"]
                            ops = pool.tile([P, C], F32, tag=ptag,
                                            name=f"o{ch}_{it}")
                            for u in range(2):
                                nc.tensor.matmul(
                                    ops[:],
                                    aoT[:, 2 * u:2 * u + 2,
                                        it * P:(it + 1) * P],
                                    h8[:, 2 * u:2 * u + 2, :],
                                    perf_mode=DR, start=(u == 0),
                                    stop=False)
                            nc.tensor.matmul(
                                ops[:],
                                d_row[0:1, it * P:(it + 1) * P],
                                rrow[:], start=False, stop=True)
                            if last:
                                xr = xrs[it]
                            else:
                                xr = xres.tile([P, C], F16, tag="xr",
                                               name=f"xr{ch}_{it}")
                                nc.sync.dma_start(xr[:], xr_t[ch * CO + it])
                            ot = ostage.tile([P, C], F16, tag="ot",
                                             name=f"ot{ch}_{it}")
                            nc.vector.scalar_tensor_tensor(
                                ot[:], ops[:], d_inv[:, it:it + 1], xr[:],
                                MULT, ADD)
                            nc.sync.dma_start(out_t[ch * CO + it], ot[:])
                        return p

                    def noop():
                        pass

                    return [p0, noop, p2, mk_it(0), mk_it(1), mk_it(2),
                            mk_it(3)]

                tail = []
                for ch in range(NCH):
                    i0 = ch * ICH
                    avs = [av_ps.tile([P, ICH], F32, tag=f"av{i}",
                                      name=f"av{ch}_{i}")
                           for i in range(CO)]
                    acc_a = accp.tile([P, ICH], F32, tag="acc_a",
                                      name=f"acca{ch}")
                    acc_b = accp.tile([P, ICH], F32, tag="acc_b",
                                      name=f"accb{ch}")

                    def scores(j, ex, jj, i0=i0, acc_a=acc_a, acc_b=acc_b,
                               ch=ch):
                        sps = sps_ps.tile([P, ICH], F32, tag="sps",
                                          name=f"sps{ch}_{j}")
                        for u in range(2):
                            nc.tensor.matmul(
                                sps[:],
                                xT8[:, 2 * u:2 * u + 2, j * P:(j + 1) * P],
                                apT8[:, 2 * u:2 * u + 2, i0:i0 + ICH],
                                perf_mode=DR, start=(u == 0), stop=(u == 1))
                        nc.scalar.activation(ex[:, jj, :], sps[:], AF.Exp,
                                             bias=shift_col, scale=SM)
                        if jj == 0:
                            if j == 0:
                                nc.vector.tensor_copy(acc_a[:], ex[:, 0, :])
                            else:
                                nc.vector.tensor_add(acc_a[:], acc_a[:],
                                                     ex[:, 0, :])
                        else:
                            if j == 1:
                                nc.gpsimd.tensor_copy(acc_b[:], ex[:, 1, :])
                            else:
                                nc.gpsimd.tensor_add(acc_b[:], acc_b[:],
                                                     ex[:, 1, :])

                    def av_mms(t, ex, avs=avs):
                        for cs in range(CO):
                            nc.tensor.matmul(
                                avs[cs][:],
                                x8i[:, t, :, cs * P:(cs + 1) * P],
                                ex[:],
                                perf_mode=DR, start=(t == 0),
                                stop=(t == JT // 2 - 1))

                    lag = 1 if ch == NCH - 1 else LAG
                    exs = {}
                    for t in range(JT // 2):
                        ex = expp.tile([P, 2, ICH], F8, tag="ex",
                                       name=f"ex{ch}_{t}")
                        exs[t] = ex
                        scores(2 * t, ex, 0)
                        scores(2 * t + 1, ex, 1)
                        if 2 <= t <= len(tail) + 1:
                            tail[t - 2]()
                        if t >= lag:
                            av_mms(t - lag, exs.pop(t - lag))
                    for t in range(JT // 2 - lag, JT // 2):
                        av_mms(t, exs.pop(t))
                    tail = make_tail(ch, avs, acc_a, acc_b,
                                     last=(ch == NCH - 1))

                def warm_tail(n, tag):
                    w = sps_ps.tile([P, P], F32, tag="sps", name=tag)
                    for wi in range(n):
                        nc.tensor.matmul(w[:], ones8[:], ones8[:],
                                         perf_mode=DR,
                                         start=(wi == 0), stop=(wi == n - 1),
                                         skip_group_check=True)

                for i, piece in enumerate(tail):
                    piece()
                    if i in (0, 2):
                        warm_tail(8, f"wt{i}")

    nc.compile()
    return nc


_NC = None


def _get_nc():
    global _NC
    if _NC is None:
        _NC = build_nc()
    return _NC


def make_in_maps(x, gn_gamma, gn_beta, wq, bq, wk, bk, wv, bv, wo, bo):
    x4 = np.asarray(x, np.float32).reshape(B, N, C)
    wq, bq = np.asarray(wq, np.float32), np.asarray(bq, np.float32)
    wk, bk = np.asarray(wk, np.float32), np.asarray(bk, np.float32)
    wv, bv = np.asarray(wv, np.float32), np.asarray(bv, np.float32)
    wo, bo = np.asarray(wo, np.float32), np.asarray(bo, np.float32)

    def wlay(w):
        return np.asarray(w, np.float32).reshape(CO, P, C).transpose(1, 0, 2)

    Gm = wq @ wk.T
    Hm = wv @ wo
    uq = bq @ wk.T
    rhost = (bv @ wo + bo) * (WS * AOS)

    rows = np.zeros((1, 4 * C), np.float32)
    for i, v in enumerate((gn_gamma, gn_beta)):
        rows[0, i * C:(i + 1) * C] = np.asarray(v, np.float32)
    rows[0, 2 * C:3 * C] = uq
    rows[0, 3 * C:4 * C] = rhost
    cst = np.zeros((P, 4), np.float32)
    cst[:, 0] = 1.0
    cst[:, 1] = 1.0
    cst[:, 2] = -SHIFT
    cst[:, 3] = EPS
    common = dict(
        g16=np.ascontiguousarray(wlay(Gm).astype(np.float16)),
        h16=np.ascontiguousarray(wlay(Hm).astype(np.float16)),
        h8=np.ascontiguousarray((WS * wlay(Hm)).astype(F8NP)),
        rows=rows, cst=cst,
    )
    in_maps = []
    for c in range(N_CORES):
        b, h = c // 2, c % 2
        own = x4[b, h * HALF:(h + 1) * HALF]
        other = x4[b, (1 - h) * HALF:(2 - h) * HALF]
        xp = np.concatenate([own, other], axis=0)        # [N, C]
        xp8 = xp.astype(F8NP)
        xT8 = np.ascontiguousarray(                      # [P, CO, N] p-major
            xp8.T.reshape(CO, P, N).transpose(1, 0, 2))
        x8i = np.ascontiguousarray(                      # [P, RT, 2C] p-major
            xp8.reshape(RT, 2, P, C).transpose(2, 0, 1, 3)
               .reshape(P, RT, 2 * C))
        xr16 = np.ascontiguousarray(own.astype(np.float16))
        in_maps.append(dict(xT8=xT8, x8i=x8i, xr16=xr16, **common))
    return in_maps


def assemble(results):
    out = np.empty((B, N, C), np.float32)
    for c in range(N_CORES):
        b, h = c // 2, c % 2
        out[b, h * HALF:(h + 1) * HALF] = results[c]["out"].astype(np.float32)
    return out.reshape(B, 64, 64, C)


def kernel(**inputs):
    nc = _get_nc()
    in_maps = make_in_maps(**inputs)
    res = run_bass_kernel_spmd(nc, in_maps, list(range(N_CORES)))
    return assemble(res.results)
